# revision 49
# baseline (speedup 1.0000x reference)
"""BitNet transformer block on 8 Trainium2 NeuronCores (Bass/Tile SPMD).

v3: same sharding as v2 (head-parallel attention with A2A to token-parallel,
pair TP-2 MLP), restructured for collective/compute overlap:
 - wo/xo/o_my prefetched during attention; o_proj runs as two token halves,
   half0 right after attention (A2A_lo already landed), half1 after A2A_hi.
 - ln2 + pair-AllGather chunked per token half, fired as each o_proj half
   completes; MLP gate/up weights chunk-0 prefetched during o_proj.
 - down-proj ReduceScatter in 8 chunks of 2 f-tiles; outD DMAs issued after
   the last RS trigger so the sync queue never blocks next-chunk weights.
 - engine rebalance: ln1 squares spread vector/gpsimd/scalar, MLP square on
   scalar so gpsimd stays trigger-only while collectives are in flight.
"""

import sys

import numpy as np

try:
    import concourse.bass as bass  # noqa: F401
except Exception:  # pragma: no cover
    sys.path.insert(0, "/opt/trn_rl_repo")

import ml_dtypes
import concourse.bass as bass
import concourse.mybir as mybir
import concourse.tile as tile
from concourse import bacc
from concourse.bass_utils import run_bass_kernel_spmd

FP32 = mybir.dt.float32
BF16 = mybir.dt.bfloat16
FP8 = mybir.dt.float8e4
BF = ml_dtypes.bfloat16
F8 = ml_dtypes.float8_e4m3

ALPHA = 0.7
EPS = 1e-5
NH = 16          # query heads
NKV = 4          # kv heads
D = 128          # head dim
H = 2048         # hidden
I_TOT = 8192     # mlp intermediate
S = 2048         # sequence
NC = 8           # cores
P = 128
HT = H // P      # 16 hidden tiles
HT2 = HT // 2    # 8 hidden tile-pairs (fp8 DoubleRow)
B = S // P       # 16 token blocks
I_LOC = I_TOT // 2    # 4096 intermediate per core (TP-2)
IT = I_LOC // P       # 32 inter tiles per core
TOK = 256             # tokens owned per core (2 blocks)
PTOK = 512            # tokens owned per pair
DR = mybir.MatmulPerfMode.DoubleRow

_CACHE = {}


def _build_program():
    nc = bacc.Bacc("TRN2", target_bir_lowering=False, debug=False, num_devices=NC)
    AF = mybir.ActivationFunctionType
    ALU = mybir.AluOpType
    rg_all = [list(range(NC))]
    rg_pair = [[2 * j, 2 * j + 1] for j in range(NC // 2)]

    # ---------------- inputs ----------------
    def dram_in(name, shape, dt=FP32):
        return nc.dram_tensor(name, shape, dt, kind="ExternalInput")

    xT_f = dram_in("xT_f", [P, HT, S], FP8)           # fp8 x^T ALL tokens (ln1 only)
    xT_own = dram_in("xT_own", [P, HT, TOK])          # fp32 x^T own cols (residual)
    cos_f = dram_in("cos_f", [P, S], BF16)
    sin_f = dram_in("sin_f", [P, S], BF16)
    wq_in = dram_in("wq", [P, 2, HT2, 2, P], FP8)     # my 2 heads [p,f,b,i,m]
    wk_in = dram_in("wk", [P, HT2, 2, P], FP8)        # my kv head
    wv_in = dram_in("wv", [P, HT2, 2, P], FP8)
    wo_in = dram_in("wo", [HT, P, HT, P], FP8)
    wg_in = dram_in("wg", [IT // 8, P, 8, HT2, 2, P], FP8)  # [g,p,f8,b,i,m]
    wu_in = dram_in("wu", [IT // 8, P, 8, HT2, 2, P], FP8)
    wd_in = dram_in("wd", [HT, P, IT // 2, 2, P], FP8)  # [fo,p,b,i,m]
    aq_in = dram_in("aq", [P, 2])
    ak_in = dram_in("ak", [P, 1])
    av_in = dram_in("av", [P, 1])
    ao_in = dram_in("ao", [P, HT])
    ag_in = dram_in("ag", [P, IT])
    au_in = dram_in("au", [P, IT])
    ad_in = dram_in("ad", [P, HT])
    rT_in = dram_in("rT", [P, P], BF16)               # rope rotate-half perm^T
    tril_in = dram_in("tril2", [P, TOK], BF16)        # [k, q] keep k<=q, both heads
    dmka_in = dram_in("dmka", [P, 4 * P], BF16)       # diag kb=2m: [tri|1|tri|1]
    dmkb_in = dram_in("dmkb", [P, 4 * P], BF16)       # diag kb=2m+1: [0|tri|0|tri]
    iden_in = dram_in("iden", [P, P], BF16)           # identity for PE transpose
    iden8_in = dram_in("iden8", [P, P], FP8)          # fp8 identity
    ones_f_in = dram_in("ones_f", [P, P])             # fp32 ones
    ones_b_in = dram_in("ones_b", [P, 1], BF16)       # bf16 ones column
    ones2_in = dram_in("ones2", [P, 2, 16], FP8)      # fp8 ones (DR lps lhsT)
    ones1_in = dram_in("ones1", [P, 16], FP8)         # fp8 ones (lps lhsT)
    invh_b_in = dram_in("invh_b", [P, 1], BF16)       # bf16 1/H column
    invh2_in = dram_in("invh2", [P, 2, 16], FP8)      # fp8 2^-9 (DR ssq lhsT)

    xmidT = nc.dram_tensor("xmidT", [P, HT, TOK], FP32, kind="ExternalOutput")
    outD = nc.dram_tensor("outD", [8, 2, P, PTOK], BF16, kind="ExternalOutput")
    outD2 = nc.dram_tensor("outD2", [8, P, PTOK], BF16, kind="ExternalOutput")

    a2a_lo_in = nc.dram_tensor("a2a_lo_in", [NC, P, 2, P], FP8)
    a2a_lo_out = nc.dram_tensor("a2a_lo_out", [NC, P, 2, P], FP8)
    a2a_hi_in = nc.dram_tensor("a2a_hi_in", [NC, P, 2, P], FP8)
    a2a_hi_out = nc.dram_tensor("a2a_hi_out", [NC, P, 2, P], FP8)
    # pair exchanges ride 2-rank AllGather (faster than 2-rank RS per byte).
    # down-sum, minimal wire: per-core wd/ad fo order is [mine, partner's]
    # (host permutes in_maps), slot-1 partial (partner's fo) is gathered;
    # final = (out[0]+out[1]) - sent + kept — sent cancels bit-exact, so the
    # program never needs to know its own pair parity.
    agx_in = [nc.dram_tensor(f"agx_in_{h}", [P, HT, P], FP8) for h in range(2)]
    agx_out = [nc.dram_tensor(f"agx_out_{h}", [2, P, HT, P], FP8)
               for h in range(2)]
    rs_in = [nc.dram_tensor(f"rs_in_{c}", [P, PTOK], BF16) for c in range(8)]
    rs_out = [nc.dram_tensor(f"rs_out_{c}", [2, P, PTOK], BF16)
              for c in range(8)]

    with tile.TileContext(nc) as tc:
        const = tc.alloc_tile_pool(name="const", bufs=1)
        ones_f = const.tile([P, P], FP32)
        ones_b = const.tile([P, 1], BF16)
        ones2 = const.tile([P, 2, 16], FP8)
        ones1 = const.tile([P, 16], FP8)
        invh_b = const.tile([P, 1], BF16)
        invh2 = const.tile([P, 2, 16], FP8)
        rT = const.tile([P, P], BF16)
        iden = const.tile([P, P], BF16)
        iden8 = const.tile([P, P], FP8)
        tril2 = const.tile([P, TOK], BF16)
        dmka = const.tile([P, 4 * P], BF16)
        dmkb = const.tile([P, 4 * P], BF16)
        aq = const.tile([P, 2], FP32)
        ak = const.tile([P, 1], FP32)
        av = const.tile([P, 1], FP32)
        ao = const.tile([P, HT], FP32)
        ag = const.tile([P, IT], FP32)
        au = const.tile([P, IT], FP32)
        ad = const.tile([P, HT], FP32)
        # order: tiny tiles needed by the first ssq/projection chain first;
        # late-use scale tiles (aq..ad) last so they don't delay chunk 0.
        for dst, src in [(invh2, invh2_in), (ones_f, ones_f_in), (rT, rT_in),
                         (iden, iden_in), (ones2, ones2_in), (ones1, ones1_in),
                         (ones_b, ones_b_in), (invh_b, invh_b_in),
                         (iden8, iden8_in), (tril2, tril_in),
                         (dmka, dmka_in), (dmkb, dmkb_in),
                         (aq, aq_in), (ak, ak_in), (av, av_in), (ao, ao_in),
                         (ag, ag_in), (au, au_in), (ad, ad_in)]:
            nc.sync.dma_start(dst[:], src[:])

        midpool = tc.alloc_tile_pool(name="midpool", bufs=1)
        x_mid = midpool.tile([P, HT, TOK], FP32)
        xopool = tc.alloc_tile_pool(name="xopool", bufs=1)
        xo = xopool.tile([P, HT, TOK], FP32)
        omypool = tc.alloc_tile_pool(name="omypool", bufs=1)
        o_my = omypool.tile([P, 2, HT, P], FP8)     # post-A2A: [half][head][tok]
        wores = tc.alloc_tile_pool(name="wo_res", bufs=1)
        wo_all = wores.tile([P, HT, HT, P], FP8)
        qkvpool = tc.alloc_tile_pool(name="qkvpool", bufs=1)
        q_my = qkvpool.tile([P, 2, S], BF16)         # my 2 heads, all tokens
        k_my = qkvpool.tile([P, B, P], BF16)         # my kv head [d, blk, tok]
        v_my = qkvpool.tile([P, B, P], FP8)          # my kv head [tok, blk, d]

        # ====== phase 1: ln1 (all tokens, chunked) + q/k/v TP projections ======
        CH4 = 512
        with tc.tile_pool(name="xc_pool", bufs=2) as xcp, \
             tc.tile_pool(name="p1sb", bufs=2) as p1sb, \
             tc.tile_pool(name="p1ps", bufs=1, space="PSUM") as p1ps, \
             tc.tile_pool(name="p2ps", bufs=2, space="PSUM") as p2ps, \
             tc.tile_pool(name="rot_ps", bufs=2, space="PSUM") as rot_ps, \
             tc.tile_pool(name="vt_ps", bufs=2, space="PSUM") as vt_ps, \
             tc.tile_pool(name="p2sb", bufs=2) as p2sb, \
             tc.tile_pool(name="cs_pool", bufs=2) as csp, \
             tc.tile_pool(name="wres", bufs=1) as wres:
            # chunk-0 x lands before the projection weights: the ssq square
            # chain (vector/gpsimd/scalar) only needs x, weights are needed
            # a few microseconds later at the first q matmul.
            wq_sb = wres.tile([P, 2, HT2, 2, P], FP8)
            wk_sb = wres.tile([P, HT2, 2, P], FP8)
            wv_sb = wres.tile([P, HT2, 2, P], FP8)
            for c4 in range(4):
                tsl = slice(c4 * CH4, (c4 + 1) * CH4)
                xc = xcp.tile([P, HT, CH4], FP8, name="xc")
                # split the 1MB chunk across both DMA queues so the first
                # hidden tiles land sooner (the ssq chain consumes in order)
                nc.scalar.dma_start(xc[:, 0:HT // 2, :], xT_f[:, 0:HT // 2, tsl])
                nc.sync.dma_start(xc[:, HT // 2:, :], xT_f[:, HT // 2:, tsl])
                if c4 == 0:
                    nc.scalar.dma_start(wq_sb[:], wq_in[:])
                    nc.scalar.dma_start(wk_sb[:], wk_in[:])
                    nc.scalar.dma_start(wv_sb[:], wv_in[:])
                cfc = csp.tile([P, CH4], BF16, name="cfc")
                nc.sync.dma_start(cfc[:], cos_f[:, tsl])
                sfc = csp.tile([P, CH4], BF16, name="sfc")
                nc.sync.dma_start(sfc[:], sin_f[:, tsl])
                # rmsnorm scale from fp8 x; applied per-token at PSUM evict.
                # squares spread across vector/gpsimd/scalar (DVE was the
                # phase-1 co-bottleneck).
                ssq = p1ps.tile([16, CH4], FP32, name="ssq")
                for b in range(HT2):
                    sq2 = p1sb.tile([P, 2, CH4], FP8, name="sq2")
                    for j in range(2):
                        k = (2 * b + j) % 3
                        if k == 0:
                            nc.vector.tensor_mul(sq2[:, j, :], xc[:, 2 * b + j, :],
                                                 xc[:, 2 * b + j, :])
                        elif k == 1:
                            nc.gpsimd.tensor_mul(sq2[:, j, :], xc[:, 2 * b + j, :],
                                                 xc[:, 2 * b + j, :])
                        else:
                            nc.scalar.activation(sq2[:, j, :], xc[:, 2 * b + j, :],
                                                 AF.Square)
                    nc.tensor.matmul(ssq[:], invh2[:], sq2[:],
                                     start=(b == 0), stop=(b == HT2 - 1),
                                     perf_mode=DR)
                ssb = p1sb.tile([1, CH4], FP32, name="ssb")
                nc.scalar.activation(ssb[:], ssq[0:1, :], AF.Copy)
                msb = p1ps.tile([P, CH4], FP32, name="msb")
                nc.tensor.matmul(msb[:], ones_f[0:1, :], ssb[:],
                                 start=True, stop=True)
                rec = p1sb.tile([P, CH4], FP32, name="rec")
                nc.vector.reciprocal_approx_fast(rec[:], msb[:])
                rsq_bc = p1sb.tile([P, CH4], FP32, name="rsq_bc", tag="rsq_bc")
                nc.scalar.activation(rsq_bc[:], rec[:], AF.Sqrt, scale=4.0)
                # q: my 2 heads (fp8 DoubleRow over hidden pairs)
                for f in range(2):
                    ps = p2ps.tile([P, CH4], FP32, name="pps")
                    for b in range(HT2):
                        nc.tensor.matmul(ps[:], wq_sb[:, f, b, :, :],
                                         xc[:, 2 * b:2 * b + 2, :],
                                         start=(b == 0), stop=(b == HT2 - 1),
                                         perf_mode=DR)
                    qs = p2sb.tile([P, CH4], BF16, name="qs")
                    nc.vector.scalar_tensor_tensor(qs[:], ps[:], aq[:, f:f + 1],
                                                   rsq_bc[:], ALU.mult, ALU.mult)
                    rot = rot_ps.tile([P, CH4], FP32, name="rot")
                    nc.tensor.matmul(rot[:], rT[:], qs[:], start=True, stop=True)
                    t1 = p2sb.tile([P, CH4], BF16, name="t1")
                    nc.vector.tensor_mul(t1[:], rot[:], sfc[:])
                    t2 = p2sb.tile([P, CH4], BF16, name="t2")
                    nc.vector.tensor_mul(t2[:], qs[:], cfc[:])
                    nc.gpsimd.tensor_add(q_my[:, f, tsl], t1[:], t2[:])
                # k: my kv head
                ps = p2ps.tile([P, CH4], FP32, name="pps")
                for b in range(HT2):
                    nc.tensor.matmul(ps[:], wk_sb[:, b, :, :],
                                     xc[:, 2 * b:2 * b + 2, :],
                                     start=(b == 0), stop=(b == HT2 - 1),
                                     perf_mode=DR)
                ks = p2sb.tile([P, CH4], BF16, name="qs")
                nc.vector.scalar_tensor_tensor(ks[:], ps[:], ak[:, 0:1],
                                               rsq_bc[:], ALU.mult, ALU.mult)
                rot = rot_ps.tile([P, CH4], FP32, name="rot")
                nc.tensor.matmul(rot[:], rT[:], ks[:], start=True, stop=True)
                t1 = p2sb.tile([P, CH4], BF16, name="t1")
                nc.vector.tensor_mul(t1[:], rot[:], sfc[:])
                t2 = p2sb.tile([P, CH4], BF16, name="t2")
                nc.vector.tensor_mul(t2[:], ks[:], cfc[:])
                nc.gpsimd.tensor_add(
                    k_my[:, 4 * c4:4 * c4 + 4, :].rearrange("p b t -> p (b t)"),
                    t1[:], t2[:])
                # v: my kv head, then PE-transpose to [tok, d] (fp8)
                ps = p2ps.tile([P, CH4], FP32, name="pps")
                for b in range(HT2):
                    nc.tensor.matmul(ps[:], wv_sb[:, b, :, :],
                                     xc[:, 2 * b:2 * b + 2, :],
                                     start=(b == 0), stop=(b == HT2 - 1),
                                     perf_mode=DR)
                vtv = p2sb.tile([P, CH4], BF16, name="vtv")
                nc.vector.scalar_tensor_tensor(vtv[:], ps[:], av[:, 0:1],
                                               rsq_bc[:], ALU.mult, ALU.mult)
                for j in range(4):
                    vtp = vt_ps.tile([P, P], BF16, name="vtp")
                    nc.tensor.transpose(vtp[:], vtv[:, j * P:(j + 1) * P], iden[:])
                    nc.vector.tensor_copy(v_my[:, 4 * c4 + j, :], vtp[:])

        # ========= prefetch for phase 3 (overlaps attention on DMA) =========
        for g in range(4):
            eng = nc.sync if g % 2 == 0 else nc.scalar
            eng.dma_start(wo_all[:, 4 * g:4 * (g + 1), :, :],
                          wo_in[4 * g:4 * (g + 1)].rearrange("f p k m -> p f k m"))
        nc.sync.dma_start(xo[:], xT_own[:])

        # ====== phase 2: attention (two consecutive query blocks fused) ======
        # q blocks {2m, 2m+1} processed together: FD=512 matmuls (columns
        # [h, qb, t]); the diagonal kb pair uses mask constants
        # dmka = [tri|1|tri|1] (kb=2m), dmkb = [0|tri|0|tri] (kb=2m+1).
        with tc.tile_pool(name="a_ps", bufs=2, space="PSUM") as a_ps, \
             tc.tile_pool(name="o_ps", bufs=2, space="PSUM") as o_ps, \
             tc.tile_pool(name="lbc_ps", bufs=1, space="PSUM") as lbc_ps, \
             tc.tile_pool(name="a_sb", bufs=3) as a_sb:
            for m in range(8):
                qb0 = 2 * m
                q_pair = q_my[:, :, qb0 * P:(qb0 + 2) * P]   # [P, 2, 256]
                ops = o_ps.tile([P, 2, 2, P], FP32, name="ops")   # [d][h][q][t]
                lps = lbc_ps.tile([16, 4 * P], FP32, name="lps")
                for g in range(m + 1):
                    kb0 = 2 * g
                    first, diag = g == 0, g == m
                    sps = a_ps.tile([P, 2, 4 * P], FP32, name="sps")
                    for j in range(2):
                        nc.tensor.matmul(sps[:, j, :], k_my[:, kb0 + j, :],
                                         q_pair, start=True, stop=True)
                    pm2 = a_sb.tile([P, 2, 4 * P], FP8, name="pm2")
                    if diag:
                        pmd = a_sb.tile([P, 2, 4 * P], BF16, name="pmd")
                        nc.scalar.activation(
                            pmd[:].rearrange("p a t -> p (a t)"),
                            sps[:].rearrange("p a t -> p (a t)"), AF.Exp)
                        nc.vector.tensor_mul(pm2[:, 0, :], pmd[:, 0, :], dmka[:])
                        nc.vector.tensor_mul(pm2[:, 1, :], pmd[:, 1, :], dmkb[:])
                    else:
                        nc.scalar.activation(
                            pm2[:].rearrange("p a t -> p (a t)"),
                            sps[:].rearrange("p a t -> p (a t)"), AF.Exp)
                    nc.tensor.matmul(lps[:], ones2[:], pm2[:],
                                     start=first, stop=diag, perf_mode=DR)
                    nc.tensor.matmul(ops[:].rearrange("p h q t -> p (h q t)"),
                                     v_my[:, kb0:kb0 + 2, :], pm2[:],
                                     start=first, stop=diag, perf_mode=DR)
                lsb = a_sb.tile([1, 4 * P], FP32, name="lsb")
                nc.scalar.activation(lsb[:], lps[0:1, :], AF.Copy)
                bca = lbc_ps.tile([P, 4 * P], FP32, name="bca")
                nc.tensor.matmul(bca[:], ones_f[0:1, :], lsb[:],
                                 start=True, stop=True)
                linv = a_sb.tile([P, 4 * P], FP32, name="linv")
                nc.vector.reciprocal_approx_fast(linv[:], bca[:])
                osb = a_sb.tile([P, 2, 2, P], FP8, name="osb")
                nc.vector.tensor_mul(
                    osb[:].rearrange("p h q t -> p (h q t)"),
                    ops[:].rearrange("p h q t -> p (h q t)"), linv[:])
                for j in range(2):
                    qb = qb0 + j
                    r_dst = min(qb, 15 - qb)
                    dst = a2a_lo_in if qb < 8 else a2a_hi_in
                    nc.sync.dma_start(dst[r_dst][:], osb[:, :, j, :])
                if m == 3:
                    nc.gpsimd.collective_compute(
                        "AllToAll", ALU.bypass, ins=[a2a_lo_in[:]],
                        outs=[a2a_lo_out[:]], replica_groups=rg_all)
                if m == 5:
                    # A2A_lo has landed by now; pull the low-half heads in
                    # (one strided DMA) while the tail query blocks compute.
                    nc.sync.dma_start(
                        o_my[:, 0].rearrange("p (j h) t -> p j h t", h=2),
                        a2a_lo_out[:].rearrange("j p h t -> p j h t"))
            nc.gpsimd.collective_compute(
                "AllToAll", ALU.bypass, ins=[a2a_hi_in[:]],
                outs=[a2a_hi_out[:]], replica_groups=rg_all)
        qkvpool.release()

        # ===== phase 3: o_proj + residual + ln2 (token halves) + pair-AG =====
        # MLP pools allocated early so gate/up chunk 0 prefetches during o_proj
        h2cp = tc.alloc_tile_pool(name="h2c_pool", bufs=1)
        h2c = h2cp.tile([P, HT, PTOK], FP8)
        mp = tc.alloc_tile_pool(name="m_pool", bufs=1)
        m_all = mp.tile([P, IT, PTOK], FP8)
        wgup = tc.alloc_tile_pool(name="wgu_pool", bufs=2)
        wtg = [None] * 4
        wtu = [None] * 4
        wtg[0] = wgup.tile([P, 8, HT2, 2, P], FP8, name="wtg8")
        nc.sync.dma_start(wtg[0][:], wg_in[0])
        wtu[0] = wgup.tile([P, 8, HT2, 2, P], FP8, name="wtu8")
        nc.scalar.dma_start(wtu[0][:], wu_in[0])

        with tc.tile_pool(name="p5ps", bufs=2, space="PSUM") as p5ps, \
             tc.tile_pool(name="p5sb", bufs=3) as p5sb, \
             tc.tile_pool(name="h2h_pool", bufs=1) as h2hp:
            h2h = h2hp.tile([P, HT, TOK], FP8)
            # high-half heads: sync queue waits on A2A_hi while PE runs half 0
            nc.sync.dma_start(
                o_my[:, 1].rearrange("p (j h) t -> p j h t", h=2),
                a2a_hi_out[:].rearrange("j p h t -> p j h t"))
            ssq2 = p5ps.tile([16, TOK], FP32, name="ssq2")
            for half in range(2):
                csl = slice(half * P, (half + 1) * P)
                # ln2 sum-of-squares interleaved into the o_proj f-loop at
                # pair granularity so the PE never drains waiting on the
                # DVE/ACT square chain after the last f-tile.
                for f in range(HT):
                    ps = p5ps.tile([P, P], FP32, name="ops5")
                    for kt in range(HT):
                        nc.tensor.matmul(ps[:], wo_all[:, f, kt, :],
                                         o_my[:, half, kt, :],
                                         start=(kt == 0), stop=(kt == HT - 1))
                    nc.vector.scalar_tensor_tensor(
                        x_mid[:, f, csl], ps[:], ao[:, f:f + 1],
                        xo[:, f, csl], ALU.mult, ALU.add)
                    if f % 2 == 1:
                        b = f // 2
                        sq2 = p5sb.tile([P, 2, P], FP8, name="sq2h")
                        nc.vector.tensor_mul(sq2[:, 0, :],
                                             x_mid[:, 2 * b, csl],
                                             x_mid[:, 2 * b, csl])
                        nc.scalar.activation(sq2[:, 1, :],
                                             x_mid[:, 2 * b + 1, csl],
                                             AF.Square)
                        nc.tensor.matmul(ssq2[:, csl], invh2[:], sq2[:],
                                         start=(b == 0), stop=(b == HT2 - 1),
                                         perf_mode=DR)
                ssb2 = p5sb.tile([1, P], FP32, name="ssb2")
                nc.scalar.activation(ssb2[:], ssq2[0:1, csl], AF.Copy)
                msb2 = p5ps.tile([P, P], FP32, name="msb2")
                nc.tensor.matmul(msb2[:], ones_f[0:1, :], ssb2[:],
                                 start=True, stop=True)
                rec2 = p5sb.tile([P, P], FP32, name="rec2")
                nc.vector.reciprocal_approx_fast(rec2[:], msb2[:])
                rsq2 = p5sb.tile([P, P], FP32, name="rsq2")
                nc.scalar.activation(rsq2[:], rec2[:], AF.Sqrt, scale=4.0)
                for kt in range(HT):
                    # half1's gpsimd muls sit behind the AG0 trigger; AG0
                    # completes well before they're needed, so even a
                    # queue-blocking trigger cannot stall them.
                    eng = nc.gpsimd if kt % 2 == 1 else nc.vector
                    eng.tensor_mul(h2h[:, kt, csl], x_mid[:, kt, csl],
                                   rsq2[:])
                nc.sync.dma_start(agx_in[half][:], h2h[:, :, csl])
                nc.gpsimd.collective_compute(
                    "AllGather", ALU.bypass, ins=[agx_in[half][:]],
                    outs=[agx_out[half][:]], replica_groups=rg_pair)
                nc.scalar.dma_start(xmidT[:, :, csl], x_mid[:, :, csl])

        # ========== phase 4: MLP (pair TP-2 over inter) + chunked RS ==========
        with tc.tile_pool(name="wd_pool", bufs=4) as wdp, \
             tc.tile_pool(name="p7ps", bufs=2, space="PSUM") as p7ps, \
             tc.tile_pool(name="p7dps", bufs=2, space="PSUM") as p7dps, \
             tc.tile_pool(name="p7sb", bufs=4) as p7sb, \
             tc.tile_pool(name="rsum", bufs=2) as rsum:
            # pair token order: [2j's 256 | (2j+1)'s 256], each = [lo128|hi128]
            for half in range(2):
                for r in range(2):
                    eng = nc.sync if r == 0 else nc.scalar
                    eng.dma_start(
                        h2c[:, :, r * TOK + half * P:r * TOK + (half + 1) * P],
                        agx_out[half][r])
            for f in range(IT):
                if f % 8 == 0 and f // 8 + 1 < 4:
                    k = f // 8 + 1
                    wtg[k] = wgup.tile([P, 8, HT2, 2, P], FP8, name="wtg8")
                    nc.sync.dma_start(wtg[k][:], wg_in[k])
                    wtu[k] = wgup.tile([P, 8, HT2, 2, P], FP8, name="wtu8")
                    nc.scalar.dma_start(wtu[k][:], wu_in[k])
                wtg8, wtu8 = wtg[f // 8], wtu[f // 8]
                gps = p7ps.tile([P, PTOK], FP32, name="gps")
                for b in range(HT2):
                    nc.tensor.matmul(gps[:], wtg8[:, f % 8, b, :, :],
                                     h2c[:, 2 * b:2 * b + 2, :],
                                     start=(b == 0), stop=(b == HT2 - 1),
                                     perf_mode=DR)
                ups = p7ps.tile([P, PTOK], FP32, name="ups")
                for b in range(HT2):
                    nc.tensor.matmul(ups[:], wtu8[:, f % 8, b, :, :],
                                     h2c[:, 2 * b:2 * b + 2, :],
                                     start=(b == 0), stop=(b == HT2 - 1),
                                     perf_mode=DR)
                gr = p7sb.tile([P, PTOK], BF16, name="gr")
                nc.vector.tensor_scalar(gr[:], gps[:], ag[:, f:f + 1], 0.0,
                                        ALU.mult, ALU.max)
                g2 = p7sb.tile([P, PTOK], BF16, name="g2")
                nc.scalar.activation(g2[:], gr[:], AF.Square)
                nc.vector.scalar_tensor_tensor(m_all[:, f, :], ups[:],
                                               au[:, f:f + 1], g2[:],
                                               ALU.mult, ALU.mult)
            # down proj in 8 chunks of 2 fo; slot 0 = MY fo (kept local),
            # slot 1 = partner's fo (gathered). dd = kept - sent goes out via
            # outD2 off the critical path; the gathered slots go out raw via
            # a single dram->dram DMA per chunk (host does the 3-way add in
            # f64: fin = out[0] + out[1] + dd — the sent tile cancels).
            def consume_rs(c):
                eng = nc.sync if c % 2 == 0 else nc.scalar
                eng.dma_start(outD[c], rs_out[c][:])

            for c in range(8):
                dn2 = [None, None]
                for j in range(2):
                    fo = 2 * c + j
                    wtd = wdp.tile([P, IT // 2, 2, P], FP8, name="wtd")
                    nc.sync.dma_start(wtd[:], wd_in[fo])
                    dps = p7dps.tile([P, PTOK], FP32, name="dps")
                    for b in range(IT // 2):
                        nc.tensor.matmul(dps[:], wtd[:, b, :, :],
                                         m_all[:, 2 * b:2 * b + 2, :],
                                         start=(b == 0), stop=(b == IT // 2 - 1),
                                         perf_mode=DR)
                    dn = p7sb.tile([P, PTOK], BF16, name=f"dn{j}")
                    nc.scalar.activation(dn[:], dps[:], AF.Copy,
                                         scale=ad[:, fo:fo + 1])
                    dn2[j] = dn
                nc.sync.dma_start(rs_in[c][:], dn2[1][:])
                dd = p7sb.tile([P, PTOK], BF16, name="dd")
                nc.vector.tensor_sub(dd[:], dn2[0][:], dn2[1][:])
                nc.scalar.dma_start(outD2[c], dd[:])
                nc.gpsimd.collective_compute(
                    "AllGather", ALU.bypass, ins=[rs_in[c][:]],
                    outs=[rs_out[c][:]], replica_groups=rg_pair)
                if c >= 2:
                    consume_rs(c - 2)
            consume_rs(6)
            consume_rs(7)
        wgup.release()
        mp.release()
        h2cp.release()
        wores.release()
        omypool.release()
        xopool.release()
        midpool.release()
        const.release()

    nc.finalize()
    return nc


def _ternary(w, fold_row=None):
    """Quantize [O, Hin] fp32 -> (ternary fp32 {-1,0,1}, absmean [O])."""
    w = np.asarray(w, dtype=np.float32)
    am = np.mean(np.abs(w), axis=1)
    t = np.sign(w) * (np.abs(w) > ALPHA * am[:, None]).astype(np.float32)
    if fold_row is not None:
        t = t * fold_row[None, :]
    return t, am


def _wlhsT(tern, n_f):
    """ternary [O, Hin] -> bf16 lhsT layout [f, p, kt, c]."""
    o, hin = tern.shape
    kt = hin // P
    assert n_f * P == o
    wT = np.ascontiguousarray(tern.T)  # [Hin, O]
    return np.ascontiguousarray(
        wT.reshape(kt, P, n_f, P).transpose(2, 1, 0, 3)).astype(BF)


def _wlhsT_dr(tern, n_f):
    """ternary [O, Hin] -> fp8 DoubleRow lhsT layout [p, f, b, i, m]:
    w[p, f, b, i, m] = ternT[128*(2b+i)+p, 128*f+m]."""
    o, hin = tern.shape
    b2 = hin // (2 * P)
    assert n_f * P == o
    wT = np.ascontiguousarray(tern.T)  # [Hin, O]
    return np.ascontiguousarray(
        wT.reshape(b2, 2, P, n_f, P).transpose(2, 3, 0, 1, 4)).astype(F8)


def _wd_layout(td_slice):
    """[H, I_loc] -> fp8 DoubleRow [fo, p, b, i, m]: wd[fo, p, b, i, m] =
    td_slice[128*fo+m, 128*(2b+i)+p]."""
    hin, iloc = td_slice.shape
    assert hin == H and iloc == I_LOC
    wT = np.ascontiguousarray(td_slice.T)  # [I_loc, H]
    return np.ascontiguousarray(
        wT.reshape(IT // 2, 2, P, HT, P).transpose(3, 2, 0, 1, 4)).astype(F8)


def _scale_tiles(a):
    """[O] -> [P, O//P] with column f = features f*128..f*128+127."""
    return np.ascontiguousarray(a.reshape(-1, P).T).astype(np.float32)


def _pcol(x2d):
    """[K, T] -> [P, K//P, T] (partition-major for direct DMA)."""
    k, t = x2d.shape
    return np.ascontiguousarray(
        x2d.reshape(k // P, P, t).transpose(1, 0, 2)).astype(np.float32)


def kernel(x, cos, sin, wq, wk, wv, wo, wg, wu, wd, ln1_w, ln2_w):
    x = np.asarray(x, dtype=np.float32)
    b, s, hdim = x.shape
    assert (b, s, hdim) == (1, S, H)

    if "nc" not in _CACHE:
        _CACHE["nc"] = _build_program()
    nc = _CACHE["nc"]

    ln1 = np.asarray(ln1_w, dtype=np.float32)
    ln2 = np.asarray(ln2_w, dtype=np.float32)

    tq, amq = _ternary(wq, fold_row=ln1)
    tk, amk = _ternary(wk, fold_row=ln1)
    tv, amv = _ternary(wv, fold_row=ln1)
    to, amo = _ternary(wo)
    tg, amg = _ternary(wg, fold_row=ln2)
    tu, amu = _ternary(wu, fold_row=ln2)
    td, amd = _ternary(wd)

    wq_h = _wlhsT_dr(tq, NH)          # [P, 16, 8, 2, P]
    wk_h = _wlhsT_dr(tk, NKV)         # [P, 4, 8, 2, P]
    wv_h = _wlhsT_dr(tv, NKV)
    wo_h = _wlhsT(to, HT).astype(F8)  # [16, P, 16, P] fp8

    aq_h = _scale_tiles(amq / np.sqrt(np.float32(D)))
    ak_h = _scale_tiles(amk)
    av_h = _scale_tiles(amv)
    ao_h = _scale_tiles(amo)
    ag_h = _scale_tiles(amg)          # [P, 64]
    au_h = _scale_tiles(amu)
    ad_h = _scale_tiles(amd)          # [P, 16]

    x2 = x[0]
    xT = np.ascontiguousarray(x2.T)
    xT_f = _pcol(xT)
    cosT = np.ascontiguousarray(np.asarray(cos, np.float32)[0, 0].T).astype(BF)
    sinT = np.ascontiguousarray(np.asarray(sin, np.float32)[0, 0].T).astype(BF)

    R = np.zeros((P, P), np.float32)
    for m in range(64):
        R[m, m + 64] = -1.0
        R[m + 64, m] = 1.0
    rT_h = np.ascontiguousarray(R.T).astype(BF)
    ones_f = np.ones((P, P), np.float32)
    ones_b = np.ones((P, 1), np.float32).astype(BF)
    invh_b = np.full((P, 1), 1.0 / H, np.float32).astype(BF)
    ones2_h = np.ones((P, 2, 16), np.float32).astype(F8)
    invh2_h = np.full((P, 2, 16), 2.0 ** -9, np.float32).astype(F8)
    ones1_h = np.ones((P, 16), np.float32).astype(F8)
    triu = np.triu(np.ones((P, P), np.float32))
    tril2_h = np.ascontiguousarray(np.concatenate([triu, triu], axis=1)).astype(BF)
    onep = np.ones((P, P), np.float32)
    zep = np.zeros((P, P), np.float32)
    dmka_h = np.ascontiguousarray(
        np.concatenate([triu, onep, triu, onep], axis=1)).astype(BF)
    dmkb_h = np.ascontiguousarray(
        np.concatenate([zep, triu, zep, triu], axis=1)).astype(BF)
    iden_h = np.eye(P, dtype=np.float32).astype(BF)

    in_maps = []
    for i in range(NC):
        blo, bhi = i, 15 - i
        own_cols = np.r_[blo * P:(blo + 1) * P, bhi * P:(bhi + 1) * P]
        kvh = i // 2
        par = i % 2
        isl = slice(par * IT, (par + 1) * IT)       # inter tile slice (TP-2)
        irow = slice(par * I_LOC, (par + 1) * I_LOC)
        # down-proj fo slot order per core: [mine (2c+par), partner's]
        fo_perm = [2 * c + (j ^ par) for c in range(8) for j in range(2)]
        in_maps.append({
            "xT_f": xT_f.astype(F8),
            "xT_own": _pcol(xT[:, own_cols]),
            "cos_f": cosT, "sin_f": sinT,
            "wq": np.ascontiguousarray(wq_h[:, 2 * i:2 * i + 2]),
            "wk": np.ascontiguousarray(wk_h[:, kvh]),
            "wv": np.ascontiguousarray(wv_h[:, kvh]),
            "wo": wo_h,
            "wg": np.ascontiguousarray(
                _wlhsT_dr(tg[irow], IT).reshape(P, 4, 8, HT2, 2, P)
                .transpose(1, 0, 2, 3, 4, 5)),
            "wu": np.ascontiguousarray(
                _wlhsT_dr(tu[irow], IT).reshape(P, 4, 8, HT2, 2, P)
                .transpose(1, 0, 2, 3, 4, 5)),
            "wd": np.ascontiguousarray(_wd_layout(td[:, irow])[fo_perm]),
            "aq": np.ascontiguousarray(aq_h[:, 2 * i:2 * i + 2]),
            "ak": np.ascontiguousarray(ak_h[:, kvh:kvh + 1]),
            "av": np.ascontiguousarray(av_h[:, kvh:kvh + 1]),
            "ao": ao_h,
            "ag": np.ascontiguousarray(ag_h[:, isl]),
            "au": np.ascontiguousarray(au_h[:, isl]),
            "ad": np.ascontiguousarray(ad_h[:, fo_perm]),
            "rT": rT_h, "tril2": tril2_h, "dmka": dmka_h, "dmkb": dmkb_h,
            "iden": iden_h,
            "iden8": iden_h.astype(F8),
            "ones_f": ones_f, "ones_b": ones_b, "invh_b": invh_b,
            "ones2": ones2_h, "ones1": ones1_h, "invh2": invh2_h,
        })

    res = run_bass_kernel_spmd(nc, in_maps, list(range(NC)))
    _CACHE["last_result"] = res

    # ---- host-side unshard: xmid residual + pair-RS output assembly ----
    out_T = np.zeros((H, S), np.float64)
    for i in range(NC):
        blo, bhi = i, 15 - i
        xm = res.results[i]["xmidT"].astype(np.float64)      # [P, HT, 256]
        xm = xm.transpose(1, 0, 2).reshape(H, TOK)
        out_T[:, blo * P:(blo + 1) * P] += xm[:, 0:P]
        out_T[:, bhi * P:(bhi + 1) * P] += xm[:, P:TOK]
    for j in range(NC // 2):
        # pair token order: [core 2j's 256 | core 2j+1's 256]
        tok_cols = np.r_[(2 * j) * P:(2 * j + 1) * P,
                         (15 - 2 * j) * P:(16 - 2 * j) * P,
                         (2 * j + 1) * P:(2 * j + 2) * P,
                         (14 - 2 * j) * P:(15 - 2 * j) * P]
        for par in range(2):
            od = res.results[2 * j + par]["outD"].astype(np.float64)  # [8,2,128,512]
            od2 = res.results[2 * j + par]["outD2"].astype(np.float64)  # [8,128,512]
            for c in range(8):
                fo = 2 * c + par
                out_T[fo * P:(fo + 1) * P][:, tok_cols] += (
                    od[c, 0] + od[c, 1] + od2[c])
    return np.ascontiguousarray(out_T.T).reshape(1, S, H).astype(np.float32)


if __name__ == "__main__":
    nc = _build_program()
    print("build OK; instructions:",
          sum(len(b.instructions) for f in nc.m.functions for b in f.blocks))


# revision 50
# speedup vs baseline: 1.0200x; 1.0200x over previous
"""BitNet transformer block on 8 Trainium2 NeuronCores (Bass/Tile SPMD).

v3: same sharding as v2 (head-parallel attention with A2A to token-parallel,
pair TP-2 MLP), restructured for collective/compute overlap:
 - wo/xo/o_my prefetched during attention; o_proj runs as two token halves,
   half0 right after attention (A2A_lo already landed), half1 after A2A_hi.
 - ln2 + pair-AllGather chunked per token half, fired as each o_proj half
   completes; MLP gate/up weights chunk-0 prefetched during o_proj.
 - down-proj ReduceScatter in 8 chunks of 2 f-tiles; outD DMAs issued after
   the last RS trigger so the sync queue never blocks next-chunk weights.
 - engine rebalance: ln1 squares spread vector/gpsimd/scalar, MLP square on
   scalar so gpsimd stays trigger-only while collectives are in flight.
"""

import sys

import numpy as np

try:
    import concourse.bass as bass  # noqa: F401
except Exception:  # pragma: no cover
    sys.path.insert(0, "/opt/trn_rl_repo")

import ml_dtypes
import concourse.bass as bass
import concourse.mybir as mybir
import concourse.tile as tile
from concourse import bacc
from concourse.bass_utils import run_bass_kernel_spmd

FP32 = mybir.dt.float32
BF16 = mybir.dt.bfloat16
FP8 = mybir.dt.float8e4
BF = ml_dtypes.bfloat16
F8 = ml_dtypes.float8_e4m3

ALPHA = 0.7
EPS = 1e-5
NH = 16          # query heads
NKV = 4          # kv heads
D = 128          # head dim
H = 2048         # hidden
I_TOT = 8192     # mlp intermediate
S = 2048         # sequence
NC = 8           # cores
P = 128
HT = H // P      # 16 hidden tiles
HT2 = HT // 2    # 8 hidden tile-pairs (fp8 DoubleRow)
B = S // P       # 16 token blocks
I_LOC = I_TOT // 2    # 4096 intermediate per core (TP-2)
IT = I_LOC // P       # 32 inter tiles per core
TOK = 256             # tokens owned per core (2 blocks)
PTOK = 512            # tokens owned per pair
DR = mybir.MatmulPerfMode.DoubleRow

_CACHE = {}


def _build_program():
    nc = bacc.Bacc("TRN2", target_bir_lowering=False, debug=False, num_devices=NC)
    AF = mybir.ActivationFunctionType
    ALU = mybir.AluOpType
    rg_all = [list(range(NC))]
    rg_pair = [[2 * j, 2 * j + 1] for j in range(NC // 2)]

    # ---------------- inputs ----------------
    def dram_in(name, shape, dt=FP32):
        return nc.dram_tensor(name, shape, dt, kind="ExternalInput")

    xT_f = dram_in("xT_f", [P, HT, S], FP8)           # fp8 x^T ALL tokens (ln1 only)
    xT_own = dram_in("xT_own", [P, HT, TOK])          # fp32 x^T own cols (residual)
    cos_f = dram_in("cos_f", [P, S], BF16)
    sin_f = dram_in("sin_f", [P, S], BF16)
    wq_in = dram_in("wq", [P, 2, HT2, 2, P], FP8)     # my 2 heads [p,f,b,i,m]
    wk_in = dram_in("wk", [P, HT2, 2, P], FP8)        # my kv head
    wv_in = dram_in("wv", [P, HT2, 2, P], FP8)
    wo_in = dram_in("wo", [HT, P, HT, P], FP8)
    wg_in = dram_in("wg", [IT // 8, P, 8, HT2, 2, P], FP8)  # [g,p,f8,b,i,m]
    wu_in = dram_in("wu", [IT // 8, P, 8, HT2, 2, P], FP8)
    wd_in = dram_in("wd", [HT, P, IT // 2, 2, P], FP8)  # [fo,p,b,i,m]
    aq_in = dram_in("aq", [P, 2])
    ak_in = dram_in("ak", [P, 1])
    av_in = dram_in("av", [P, 1])
    ao_in = dram_in("ao", [P, HT])
    ag_in = dram_in("ag", [P, IT])
    au_in = dram_in("au", [P, IT])
    ad_in = dram_in("ad", [P, HT])
    rT_in = dram_in("rT", [P, P], BF16)               # rope rotate-half perm^T
    tril_in = dram_in("tril2", [P, TOK], BF16)        # [k, q] keep k<=q, both heads
    dmka_in = dram_in("dmka", [P, 4 * P], BF16)       # diag kb=2m: [tri|1|tri|1]
    dmkb_in = dram_in("dmkb", [P, 4 * P], BF16)       # diag kb=2m+1: [0|tri|0|tri]
    iden_in = dram_in("iden", [P, P], BF16)           # identity for PE transpose
    iden8_in = dram_in("iden8", [P, P], FP8)          # fp8 identity
    ones_f_in = dram_in("ones_f", [P, P])             # fp32 ones
    ones_b_in = dram_in("ones_b", [P, 1], BF16)       # bf16 ones column
    ones2_in = dram_in("ones2", [P, 2, 16], FP8)      # fp8 ones (DR lps lhsT)
    ones1_in = dram_in("ones1", [P, 16], FP8)         # fp8 ones (lps lhsT)
    invh_b_in = dram_in("invh_b", [P, 1], BF16)       # bf16 1/H column
    invh2_in = dram_in("invh2", [P, 2, 16], FP8)      # fp8 2^-9 (DR ssq lhsT)

    xmidT = nc.dram_tensor("xmidT", [P, HT, TOK], FP32, kind="ExternalOutput")
    outD = nc.dram_tensor("outD", [8, 2, P, PTOK], BF16, kind="ExternalOutput")
    outD2 = nc.dram_tensor("outD2", [8, P, PTOK], BF16, kind="ExternalOutput")

    a2a_lo_in = nc.dram_tensor("a2a_lo_in", [NC, P, 2, P], FP8)
    a2a_lo_out = nc.dram_tensor("a2a_lo_out", [NC, P, 2, P], FP8)
    a2a_hi_in = nc.dram_tensor("a2a_hi_in", [NC, P, 2, P], FP8)
    a2a_hi_out = nc.dram_tensor("a2a_hi_out", [NC, P, 2, P], FP8)
    # pair exchanges ride 2-rank AllGather (faster than 2-rank RS per byte).
    # down-sum, minimal wire: per-core wd/ad fo order is [mine, partner's]
    # (host permutes in_maps), slot-1 partial (partner's fo) is gathered;
    # final = (out[0]+out[1]) - sent + kept — sent cancels bit-exact, so the
    # program never needs to know its own pair parity.
    agx_in = [nc.dram_tensor(f"agx_in_{h}", [P, HT, P], FP8) for h in range(2)]
    agx_out = [nc.dram_tensor(f"agx_out_{h}", [2, P, HT, P], FP8)
               for h in range(2)]
    rs_in = [nc.dram_tensor(f"rs_in_{c}", [P, PTOK], BF16) for c in range(8)]
    rs_out = [nc.dram_tensor(f"rs_out_{c}", [2, P, PTOK], BF16)
              for c in range(8)]

    with tile.TileContext(nc) as tc:
        const = tc.alloc_tile_pool(name="const", bufs=1)
        ones_f = const.tile([P, P], FP32)
        ones_b = const.tile([P, 1], BF16)
        ones2 = const.tile([P, 2, 16], FP8)
        ones1 = const.tile([P, 16], FP8)
        invh_b = const.tile([P, 1], BF16)
        invh2 = const.tile([P, 2, 16], FP8)
        rT = const.tile([P, P], BF16)
        iden = const.tile([P, P], BF16)
        iden8 = const.tile([P, P], FP8)
        tril2 = const.tile([P, TOK], BF16)
        dmka = const.tile([P, 4 * P], BF16)
        dmkb = const.tile([P, 4 * P], BF16)
        aq = const.tile([P, 2], FP32)
        ak = const.tile([P, 1], FP32)
        av = const.tile([P, 1], FP32)
        ao = const.tile([P, HT], FP32)
        ag = const.tile([P, IT], FP32)
        au = const.tile([P, IT], FP32)
        ad = const.tile([P, HT], FP32)
        # order: tiny tiles needed by the first ssq/projection chain first;
        # late-use scale tiles (aq..ad) last so they don't delay chunk 0.
        for dst, src in [(invh2, invh2_in), (ones_f, ones_f_in), (rT, rT_in),
                         (iden, iden_in), (ones2, ones2_in), (ones1, ones1_in),
                         (ones_b, ones_b_in), (invh_b, invh_b_in),
                         (iden8, iden8_in), (tril2, tril_in),
                         (dmka, dmka_in), (dmkb, dmkb_in),
                         (aq, aq_in), (ak, ak_in), (av, av_in), (ao, ao_in),
                         (ag, ag_in), (au, au_in), (ad, ad_in)]:
            nc.sync.dma_start(dst[:], src[:])

        midpool = tc.alloc_tile_pool(name="midpool", bufs=1)
        x_mid = midpool.tile([P, HT, TOK], FP32)
        xopool = tc.alloc_tile_pool(name="xopool", bufs=1)
        xo = xopool.tile([P, HT, TOK], FP32)
        omypool = tc.alloc_tile_pool(name="omypool", bufs=1)
        o_my = omypool.tile([P, HT, TOK], FP8)      # post-A2A: 16 heads x my toks
        wores = tc.alloc_tile_pool(name="wo_res", bufs=1)
        wo_all = wores.tile([P, HT, HT, P], FP8)
        qkvpool = tc.alloc_tile_pool(name="qkvpool", bufs=1)
        q_my = qkvpool.tile([P, 2, S], BF16)         # my 2 heads, all tokens
        k_my = qkvpool.tile([P, B, P], BF16)         # my kv head [d, blk, tok]
        v_my = qkvpool.tile([P, B, P], FP8)          # my kv head [tok, blk, d]

        # ====== phase 1: ln1 (all tokens, chunked) + q/k/v TP projections ======
        CH4 = 512
        with tc.tile_pool(name="xc_pool", bufs=2) as xcp, \
             tc.tile_pool(name="p1sb", bufs=2) as p1sb, \
             tc.tile_pool(name="p1ps", bufs=1, space="PSUM") as p1ps, \
             tc.tile_pool(name="p2ps", bufs=2, space="PSUM") as p2ps, \
             tc.tile_pool(name="rot_ps", bufs=2, space="PSUM") as rot_ps, \
             tc.tile_pool(name="vt_ps", bufs=2, space="PSUM") as vt_ps, \
             tc.tile_pool(name="p2sb", bufs=2) as p2sb, \
             tc.tile_pool(name="cs_pool", bufs=2) as csp, \
             tc.tile_pool(name="wres", bufs=1) as wres:
            # chunk-0 x lands before the projection weights: the ssq square
            # chain (vector/gpsimd/scalar) only needs x, weights are needed
            # a few microseconds later at the first q matmul.
            wq_sb = wres.tile([P, 2, HT2, 2, P], FP8)
            wk_sb = wres.tile([P, HT2, 2, P], FP8)
            wv_sb = wres.tile([P, HT2, 2, P], FP8)
            for c4 in range(4):
                tsl = slice(c4 * CH4, (c4 + 1) * CH4)
                xc = xcp.tile([P, HT, CH4], FP8, name="xc")
                # split the 1MB chunk across both DMA queues so the first
                # hidden tiles land sooner (the ssq chain consumes in order)
                nc.scalar.dma_start(xc[:, 0:HT // 2, :], xT_f[:, 0:HT // 2, tsl])
                nc.sync.dma_start(xc[:, HT // 2:, :], xT_f[:, HT // 2:, tsl])
                if c4 == 0:
                    nc.scalar.dma_start(wq_sb[:], wq_in[:])
                    nc.scalar.dma_start(wk_sb[:], wk_in[:])
                    nc.scalar.dma_start(wv_sb[:], wv_in[:])
                cfc = csp.tile([P, CH4], BF16, name="cfc")
                nc.sync.dma_start(cfc[:], cos_f[:, tsl])
                sfc = csp.tile([P, CH4], BF16, name="sfc")
                nc.sync.dma_start(sfc[:], sin_f[:, tsl])
                # rmsnorm scale from fp8 x; applied per-token at PSUM evict.
                # squares spread across vector/gpsimd/scalar (DVE was the
                # phase-1 co-bottleneck).
                ssq = p1ps.tile([16, CH4], FP32, name="ssq")
                for b in range(HT2):
                    sq2 = p1sb.tile([P, 2, CH4], FP8, name="sq2")
                    for j in range(2):
                        k = (2 * b + j) % 3
                        if k == 0:
                            nc.vector.tensor_mul(sq2[:, j, :], xc[:, 2 * b + j, :],
                                                 xc[:, 2 * b + j, :])
                        elif k == 1:
                            nc.gpsimd.tensor_mul(sq2[:, j, :], xc[:, 2 * b + j, :],
                                                 xc[:, 2 * b + j, :])
                        else:
                            nc.scalar.activation(sq2[:, j, :], xc[:, 2 * b + j, :],
                                                 AF.Square)
                    nc.tensor.matmul(ssq[:], invh2[:], sq2[:],
                                     start=(b == 0), stop=(b == HT2 - 1),
                                     perf_mode=DR)
                ssb = p1sb.tile([1, CH4], FP32, name="ssb")
                nc.scalar.activation(ssb[:], ssq[0:1, :], AF.Copy)
                msb = p1ps.tile([P, CH4], FP32, name="msb")
                nc.tensor.matmul(msb[:], ones_f[0:1, :], ssb[:],
                                 start=True, stop=True)
                rec = p1sb.tile([P, CH4], FP32, name="rec")
                nc.vector.reciprocal_approx_fast(rec[:], msb[:])
                rsq_bc = p1sb.tile([P, CH4], FP32, name="rsq_bc", tag="rsq_bc")
                nc.scalar.activation(rsq_bc[:], rec[:], AF.Sqrt, scale=4.0)
                # q: my 2 heads (fp8 DoubleRow over hidden pairs)
                for f in range(2):
                    ps = p2ps.tile([P, CH4], FP32, name="pps")
                    for b in range(HT2):
                        nc.tensor.matmul(ps[:], wq_sb[:, f, b, :, :],
                                         xc[:, 2 * b:2 * b + 2, :],
                                         start=(b == 0), stop=(b == HT2 - 1),
                                         perf_mode=DR)
                    qs = p2sb.tile([P, CH4], BF16, name="qs")
                    nc.vector.scalar_tensor_tensor(qs[:], ps[:], aq[:, f:f + 1],
                                                   rsq_bc[:], ALU.mult, ALU.mult)
                    rot = rot_ps.tile([P, CH4], FP32, name="rot")
                    nc.tensor.matmul(rot[:], rT[:], qs[:], start=True, stop=True)
                    t1 = p2sb.tile([P, CH4], BF16, name="t1")
                    nc.vector.tensor_mul(t1[:], rot[:], sfc[:])
                    t2 = p2sb.tile([P, CH4], BF16, name="t2")
                    nc.vector.tensor_mul(t2[:], qs[:], cfc[:])
                    nc.gpsimd.tensor_add(q_my[:, f, tsl], t1[:], t2[:])
                # k: my kv head
                ps = p2ps.tile([P, CH4], FP32, name="pps")
                for b in range(HT2):
                    nc.tensor.matmul(ps[:], wk_sb[:, b, :, :],
                                     xc[:, 2 * b:2 * b + 2, :],
                                     start=(b == 0), stop=(b == HT2 - 1),
                                     perf_mode=DR)
                ks = p2sb.tile([P, CH4], BF16, name="qs")
                nc.vector.scalar_tensor_tensor(ks[:], ps[:], ak[:, 0:1],
                                               rsq_bc[:], ALU.mult, ALU.mult)
                rot = rot_ps.tile([P, CH4], FP32, name="rot")
                nc.tensor.matmul(rot[:], rT[:], ks[:], start=True, stop=True)
                t1 = p2sb.tile([P, CH4], BF16, name="t1")
                nc.vector.tensor_mul(t1[:], rot[:], sfc[:])
                t2 = p2sb.tile([P, CH4], BF16, name="t2")
                nc.vector.tensor_mul(t2[:], ks[:], cfc[:])
                nc.gpsimd.tensor_add(
                    k_my[:, 4 * c4:4 * c4 + 4, :].rearrange("p b t -> p (b t)"),
                    t1[:], t2[:])
                # v: my kv head, then PE-transpose to [tok, d] (fp8)
                ps = p2ps.tile([P, CH4], FP32, name="pps")
                for b in range(HT2):
                    nc.tensor.matmul(ps[:], wv_sb[:, b, :, :],
                                     xc[:, 2 * b:2 * b + 2, :],
                                     start=(b == 0), stop=(b == HT2 - 1),
                                     perf_mode=DR)
                vtv = p2sb.tile([P, CH4], BF16, name="vtv")
                nc.vector.scalar_tensor_tensor(vtv[:], ps[:], av[:, 0:1],
                                               rsq_bc[:], ALU.mult, ALU.mult)
                for j in range(4):
                    vtp = vt_ps.tile([P, P], BF16, name="vtp")
                    nc.tensor.transpose(vtp[:], vtv[:, j * P:(j + 1) * P], iden[:])
                    nc.vector.tensor_copy(v_my[:, 4 * c4 + j, :], vtp[:])

        # ========= prefetch for phase 3 (overlaps attention on DMA) =========
        for g in range(4):
            eng = nc.sync if g % 2 == 0 else nc.scalar
            eng.dma_start(wo_all[:, 4 * g:4 * (g + 1), :, :],
                          wo_in[4 * g:4 * (g + 1)].rearrange("f p k m -> p f k m"))
        nc.sync.dma_start(xo[:], xT_own[:])

        # ====== phase 2: attention (two consecutive query blocks fused) ======
        # q blocks {2m, 2m+1} processed together: FD=512 matmuls (columns
        # [h, qb, t]); the diagonal kb pair uses mask constants
        # dmka = [tri|1|tri|1] (kb=2m), dmkb = [0|tri|0|tri] (kb=2m+1).
        with tc.tile_pool(name="a_ps", bufs=2, space="PSUM") as a_ps, \
             tc.tile_pool(name="o_ps", bufs=2, space="PSUM") as o_ps, \
             tc.tile_pool(name="lbc_ps", bufs=1, space="PSUM") as lbc_ps, \
             tc.tile_pool(name="a_sb", bufs=3) as a_sb:
            for m in range(8):
                qb0 = 2 * m
                q_pair = q_my[:, :, qb0 * P:(qb0 + 2) * P]   # [P, 2, 256]
                ops = o_ps.tile([P, 2, 2, P], FP32, name="ops")   # [d][h][q][t]
                lps = lbc_ps.tile([16, 4 * P], FP32, name="lps")
                for g in range(m + 1):
                    kb0 = 2 * g
                    first, diag = g == 0, g == m
                    sps = a_ps.tile([P, 2, 4 * P], FP32, name="sps")
                    for j in range(2):
                        nc.tensor.matmul(sps[:, j, :], k_my[:, kb0 + j, :],
                                         q_pair, start=True, stop=True)
                    pm2 = a_sb.tile([P, 2, 4 * P], FP8, name="pm2")
                    if diag:
                        pmd = a_sb.tile([P, 2, 4 * P], BF16, name="pmd")
                        nc.scalar.activation(
                            pmd[:].rearrange("p a t -> p (a t)"),
                            sps[:].rearrange("p a t -> p (a t)"), AF.Exp)
                        nc.vector.tensor_mul(pm2[:, 0, :], pmd[:, 0, :], dmka[:])
                        nc.vector.tensor_mul(pm2[:, 1, :], pmd[:, 1, :], dmkb[:])
                    else:
                        nc.scalar.activation(
                            pm2[:].rearrange("p a t -> p (a t)"),
                            sps[:].rearrange("p a t -> p (a t)"), AF.Exp)
                    nc.tensor.matmul(lps[:], ones2[:], pm2[:],
                                     start=first, stop=diag, perf_mode=DR)
                    nc.tensor.matmul(ops[:].rearrange("p h q t -> p (h q t)"),
                                     v_my[:, kb0:kb0 + 2, :], pm2[:],
                                     start=first, stop=diag, perf_mode=DR)
                lsb = a_sb.tile([1, 4 * P], FP32, name="lsb")
                nc.scalar.activation(lsb[:], lps[0:1, :], AF.Copy)
                bca = lbc_ps.tile([P, 4 * P], FP32, name="bca")
                nc.tensor.matmul(bca[:], ones_f[0:1, :], lsb[:],
                                 start=True, stop=True)
                linv = a_sb.tile([P, 4 * P], FP32, name="linv")
                nc.vector.reciprocal_approx_fast(linv[:], bca[:])
                osb = a_sb.tile([P, 2, 2, P], FP8, name="osb")
                nc.vector.tensor_mul(
                    osb[:].rearrange("p h q t -> p (h q t)"),
                    ops[:].rearrange("p h q t -> p (h q t)"), linv[:])
                for j in range(2):
                    qb = qb0 + j
                    r_dst = min(qb, 15 - qb)
                    dst = a2a_lo_in if qb < 8 else a2a_hi_in
                    nc.sync.dma_start(dst[r_dst][:], osb[:, :, j, :])
                if m == 3:
                    nc.gpsimd.collective_compute(
                        "AllToAll", ALU.bypass, ins=[a2a_lo_in[:]],
                        outs=[a2a_lo_out[:]], replica_groups=rg_all)
                if m == 5:
                    # A2A_lo has landed by now; pull the low-half heads in
                    # while the tail query blocks compute.
                    for j in range(NC):
                        nc.sync.dma_start(o_my[:, 2 * j:2 * j + 2, 0:P],
                                          a2a_lo_out[j])
            nc.gpsimd.collective_compute(
                "AllToAll", ALU.bypass, ins=[a2a_hi_in[:]],
                outs=[a2a_hi_out[:]], replica_groups=rg_all)
        qkvpool.release()

        # ===== phase 3: o_proj + residual + ln2 (token halves) + pair-AG =====
        # MLP pools allocated early so gate/up chunk 0 prefetches during o_proj
        h2cp = tc.alloc_tile_pool(name="h2c_pool", bufs=1)
        h2c = h2cp.tile([P, HT, PTOK], FP8)
        mp = tc.alloc_tile_pool(name="m_pool", bufs=1)
        m_all = mp.tile([P, IT, PTOK], FP8)
        wgup = tc.alloc_tile_pool(name="wgu_pool", bufs=2)
        wtg = [None] * 4
        wtu = [None] * 4
        wtg[0] = wgup.tile([P, 8, HT2, 2, P], FP8, name="wtg8")
        nc.sync.dma_start(wtg[0][:], wg_in[0])
        wtu[0] = wgup.tile([P, 8, HT2, 2, P], FP8, name="wtu8")
        nc.scalar.dma_start(wtu[0][:], wu_in[0])

        with tc.tile_pool(name="p5ps", bufs=2, space="PSUM") as p5ps, \
             tc.tile_pool(name="p5sb", bufs=3) as p5sb, \
             tc.tile_pool(name="h2h_pool", bufs=1) as h2hp:
            h2h = h2hp.tile([P, HT, TOK], FP8)
            # high-half heads: sync queue waits on A2A_hi while PE runs half 0
            for j in range(NC):
                nc.sync.dma_start(o_my[:, 2 * j:2 * j + 2, P:TOK],
                                  a2a_hi_out[j])
            ssq2 = p5ps.tile([16, TOK], FP32, name="ssq2")
            for half in range(2):
                csl = slice(half * P, (half + 1) * P)
                # ln2 sum-of-squares interleaved into the o_proj f-loop at
                # pair granularity so the PE never drains waiting on the
                # DVE/ACT square chain after the last f-tile.
                for f in range(HT):
                    ps = p5ps.tile([P, P], FP32, name="ops5")
                    for kt in range(HT):
                        nc.tensor.matmul(ps[:], wo_all[:, f, kt, :], o_my[:, kt, csl],
                                         start=(kt == 0), stop=(kt == HT - 1))
                    nc.vector.scalar_tensor_tensor(
                        x_mid[:, f, csl], ps[:], ao[:, f:f + 1],
                        xo[:, f, csl], ALU.mult, ALU.add)
                    if f % 2 == 1:
                        b = f // 2
                        sq2 = p5sb.tile([P, 2, P], FP8, name="sq2h")
                        nc.vector.tensor_mul(sq2[:, 0, :],
                                             x_mid[:, 2 * b, csl],
                                             x_mid[:, 2 * b, csl])
                        nc.scalar.activation(sq2[:, 1, :],
                                             x_mid[:, 2 * b + 1, csl],
                                             AF.Square)
                        nc.tensor.matmul(ssq2[:, csl], invh2[:], sq2[:],
                                         start=(b == 0), stop=(b == HT2 - 1),
                                         perf_mode=DR)
                ssb2 = p5sb.tile([1, P], FP32, name="ssb2")
                nc.scalar.activation(ssb2[:], ssq2[0:1, csl], AF.Copy)
                msb2 = p5ps.tile([P, P], FP32, name="msb2")
                nc.tensor.matmul(msb2[:], ones_f[0:1, :], ssb2[:],
                                 start=True, stop=True)
                rec2 = p5sb.tile([P, P], FP32, name="rec2")
                nc.vector.reciprocal_approx_fast(rec2[:], msb2[:])
                rsq2 = p5sb.tile([P, P], FP32, name="rsq2")
                nc.scalar.activation(rsq2[:], rec2[:], AF.Sqrt, scale=4.0)
                for kt in range(HT):
                    # gpsimd only before its first trigger (AG0) is enqueued:
                    # a trigger may block the queue until the CC completes.
                    eng = (nc.gpsimd if (half == 0 and kt % 2 == 1)
                           else nc.vector)
                    eng.tensor_mul(h2h[:, kt, csl], x_mid[:, kt, csl],
                                   rsq2[:])
                nc.sync.dma_start(agx_in[half][:], h2h[:, :, csl])
                nc.gpsimd.collective_compute(
                    "AllGather", ALU.bypass, ins=[agx_in[half][:]],
                    outs=[agx_out[half][:]], replica_groups=rg_pair)
                nc.scalar.dma_start(xmidT[:, :, csl], x_mid[:, :, csl])

        # ========== phase 4: MLP (pair TP-2 over inter) + chunked RS ==========
        with tc.tile_pool(name="wd_pool", bufs=4) as wdp, \
             tc.tile_pool(name="p7ps", bufs=2, space="PSUM") as p7ps, \
             tc.tile_pool(name="p7dps", bufs=2, space="PSUM") as p7dps, \
             tc.tile_pool(name="p7sb", bufs=4) as p7sb, \
             tc.tile_pool(name="rsum", bufs=2) as rsum:
            # pair token order: [2j's 256 | (2j+1)'s 256], each = [lo128|hi128]
            for half in range(2):
                for r in range(2):
                    eng = nc.sync if r == 0 else nc.scalar
                    eng.dma_start(
                        h2c[:, :, r * TOK + half * P:r * TOK + (half + 1) * P],
                        agx_out[half][r])
            for f in range(IT):
                if f % 8 == 0 and f // 8 + 1 < 4:
                    k = f // 8 + 1
                    wtg[k] = wgup.tile([P, 8, HT2, 2, P], FP8, name="wtg8")
                    nc.sync.dma_start(wtg[k][:], wg_in[k])
                    wtu[k] = wgup.tile([P, 8, HT2, 2, P], FP8, name="wtu8")
                    nc.scalar.dma_start(wtu[k][:], wu_in[k])
                wtg8, wtu8 = wtg[f // 8], wtu[f // 8]
                gps = p7ps.tile([P, PTOK], FP32, name="gps")
                for b in range(HT2):
                    nc.tensor.matmul(gps[:], wtg8[:, f % 8, b, :, :],
                                     h2c[:, 2 * b:2 * b + 2, :],
                                     start=(b == 0), stop=(b == HT2 - 1),
                                     perf_mode=DR)
                ups = p7ps.tile([P, PTOK], FP32, name="ups")
                for b in range(HT2):
                    nc.tensor.matmul(ups[:], wtu8[:, f % 8, b, :, :],
                                     h2c[:, 2 * b:2 * b + 2, :],
                                     start=(b == 0), stop=(b == HT2 - 1),
                                     perf_mode=DR)
                gr = p7sb.tile([P, PTOK], BF16, name="gr")
                nc.vector.tensor_scalar(gr[:], gps[:], ag[:, f:f + 1], 0.0,
                                        ALU.mult, ALU.max)
                g2 = p7sb.tile([P, PTOK], BF16, name="g2")
                nc.scalar.activation(g2[:], gr[:], AF.Square)
                nc.vector.scalar_tensor_tensor(m_all[:, f, :], ups[:],
                                               au[:, f:f + 1], g2[:],
                                               ALU.mult, ALU.mult)
            # down proj in 8 chunks of 2 fo; slot 0 = MY fo (kept local),
            # slot 1 = partner's fo (gathered). dd = kept - sent goes out via
            # outD2 off the critical path; the gathered slots go out raw via
            # a single dram->dram DMA per chunk (host does the 3-way add in
            # f64: fin = out[0] + out[1] + dd — the sent tile cancels).
            def consume_rs(c):
                eng = nc.sync if c % 2 == 0 else nc.scalar
                eng.dma_start(outD[c], rs_out[c][:])

            for c in range(8):
                dn2 = [None, None]
                for j in range(2):
                    fo = 2 * c + j
                    wtd = wdp.tile([P, IT // 2, 2, P], FP8, name="wtd")
                    nc.sync.dma_start(wtd[:], wd_in[fo])
                    dps = p7dps.tile([P, PTOK], FP32, name="dps")
                    for b in range(IT // 2):
                        nc.tensor.matmul(dps[:], wtd[:, b, :, :],
                                         m_all[:, 2 * b:2 * b + 2, :],
                                         start=(b == 0), stop=(b == IT // 2 - 1),
                                         perf_mode=DR)
                    dn = p7sb.tile([P, PTOK], BF16, name=f"dn{j}")
                    nc.scalar.activation(dn[:], dps[:], AF.Copy,
                                         scale=ad[:, fo:fo + 1])
                    dn2[j] = dn
                nc.sync.dma_start(rs_in[c][:], dn2[1][:])
                dd = p7sb.tile([P, PTOK], BF16, name="dd")
                nc.vector.tensor_sub(dd[:], dn2[0][:], dn2[1][:])
                nc.scalar.dma_start(outD2[c], dd[:])
                nc.gpsimd.collective_compute(
                    "AllGather", ALU.bypass, ins=[rs_in[c][:]],
                    outs=[rs_out[c][:]], replica_groups=rg_pair)
                if c >= 2:
                    consume_rs(c - 2)
            consume_rs(6)
            consume_rs(7)
        wgup.release()
        mp.release()
        h2cp.release()
        wores.release()
        omypool.release()
        xopool.release()
        midpool.release()
        const.release()

    nc.finalize()
    return nc


def _ternary(w, fold_row=None):
    """Quantize [O, Hin] fp32 -> (ternary fp32 {-1,0,1}, absmean [O])."""
    w = np.asarray(w, dtype=np.float32)
    am = np.mean(np.abs(w), axis=1)
    t = np.sign(w) * (np.abs(w) > ALPHA * am[:, None]).astype(np.float32)
    if fold_row is not None:
        t = t * fold_row[None, :]
    return t, am


def _wlhsT(tern, n_f):
    """ternary [O, Hin] -> bf16 lhsT layout [f, p, kt, c]."""
    o, hin = tern.shape
    kt = hin // P
    assert n_f * P == o
    wT = np.ascontiguousarray(tern.T)  # [Hin, O]
    return np.ascontiguousarray(
        wT.reshape(kt, P, n_f, P).transpose(2, 1, 0, 3)).astype(BF)


def _wlhsT_dr(tern, n_f):
    """ternary [O, Hin] -> fp8 DoubleRow lhsT layout [p, f, b, i, m]:
    w[p, f, b, i, m] = ternT[128*(2b+i)+p, 128*f+m]."""
    o, hin = tern.shape
    b2 = hin // (2 * P)
    assert n_f * P == o
    wT = np.ascontiguousarray(tern.T)  # [Hin, O]
    return np.ascontiguousarray(
        wT.reshape(b2, 2, P, n_f, P).transpose(2, 3, 0, 1, 4)).astype(F8)


def _wd_layout(td_slice):
    """[H, I_loc] -> fp8 DoubleRow [fo, p, b, i, m]: wd[fo, p, b, i, m] =
    td_slice[128*fo+m, 128*(2b+i)+p]."""
    hin, iloc = td_slice.shape
    assert hin == H and iloc == I_LOC
    wT = np.ascontiguousarray(td_slice.T)  # [I_loc, H]
    return np.ascontiguousarray(
        wT.reshape(IT // 2, 2, P, HT, P).transpose(3, 2, 0, 1, 4)).astype(F8)


def _scale_tiles(a):
    """[O] -> [P, O//P] with column f = features f*128..f*128+127."""
    return np.ascontiguousarray(a.reshape(-1, P).T).astype(np.float32)


def _pcol(x2d):
    """[K, T] -> [P, K//P, T] (partition-major for direct DMA)."""
    k, t = x2d.shape
    return np.ascontiguousarray(
        x2d.reshape(k // P, P, t).transpose(1, 0, 2)).astype(np.float32)


def kernel(x, cos, sin, wq, wk, wv, wo, wg, wu, wd, ln1_w, ln2_w):
    x = np.asarray(x, dtype=np.float32)
    b, s, hdim = x.shape
    assert (b, s, hdim) == (1, S, H)

    if "nc" not in _CACHE:
        _CACHE["nc"] = _build_program()
    nc = _CACHE["nc"]

    ln1 = np.asarray(ln1_w, dtype=np.float32)
    ln2 = np.asarray(ln2_w, dtype=np.float32)

    tq, amq = _ternary(wq, fold_row=ln1)
    tk, amk = _ternary(wk, fold_row=ln1)
    tv, amv = _ternary(wv, fold_row=ln1)
    to, amo = _ternary(wo)
    tg, amg = _ternary(wg, fold_row=ln2)
    tu, amu = _ternary(wu, fold_row=ln2)
    td, amd = _ternary(wd)

    wq_h = _wlhsT_dr(tq, NH)          # [P, 16, 8, 2, P]
    wk_h = _wlhsT_dr(tk, NKV)         # [P, 4, 8, 2, P]
    wv_h = _wlhsT_dr(tv, NKV)
    wo_h = _wlhsT(to, HT).astype(F8)  # [16, P, 16, P] fp8

    aq_h = _scale_tiles(amq / np.sqrt(np.float32(D)))
    ak_h = _scale_tiles(amk)
    av_h = _scale_tiles(amv)
    ao_h = _scale_tiles(amo)
    ag_h = _scale_tiles(amg)          # [P, 64]
    au_h = _scale_tiles(amu)
    ad_h = _scale_tiles(amd)          # [P, 16]

    x2 = x[0]
    xT = np.ascontiguousarray(x2.T)
    xT_f = _pcol(xT)
    cosT = np.ascontiguousarray(np.asarray(cos, np.float32)[0, 0].T).astype(BF)
    sinT = np.ascontiguousarray(np.asarray(sin, np.float32)[0, 0].T).astype(BF)

    R = np.zeros((P, P), np.float32)
    for m in range(64):
        R[m, m + 64] = -1.0
        R[m + 64, m] = 1.0
    rT_h = np.ascontiguousarray(R.T).astype(BF)
    ones_f = np.ones((P, P), np.float32)
    ones_b = np.ones((P, 1), np.float32).astype(BF)
    invh_b = np.full((P, 1), 1.0 / H, np.float32).astype(BF)
    ones2_h = np.ones((P, 2, 16), np.float32).astype(F8)
    invh2_h = np.full((P, 2, 16), 2.0 ** -9, np.float32).astype(F8)
    ones1_h = np.ones((P, 16), np.float32).astype(F8)
    triu = np.triu(np.ones((P, P), np.float32))
    tril2_h = np.ascontiguousarray(np.concatenate([triu, triu], axis=1)).astype(BF)
    onep = np.ones((P, P), np.float32)
    zep = np.zeros((P, P), np.float32)
    dmka_h = np.ascontiguousarray(
        np.concatenate([triu, onep, triu, onep], axis=1)).astype(BF)
    dmkb_h = np.ascontiguousarray(
        np.concatenate([zep, triu, zep, triu], axis=1)).astype(BF)
    iden_h = np.eye(P, dtype=np.float32).astype(BF)

    in_maps = []
    for i in range(NC):
        blo, bhi = i, 15 - i
        own_cols = np.r_[blo * P:(blo + 1) * P, bhi * P:(bhi + 1) * P]
        kvh = i // 2
        par = i % 2
        isl = slice(par * IT, (par + 1) * IT)       # inter tile slice (TP-2)
        irow = slice(par * I_LOC, (par + 1) * I_LOC)
        # down-proj fo slot order per core: [mine (2c+par), partner's]
        fo_perm = [2 * c + (j ^ par) for c in range(8) for j in range(2)]
        in_maps.append({
            "xT_f": xT_f.astype(F8),
            "xT_own": _pcol(xT[:, own_cols]),
            "cos_f": cosT, "sin_f": sinT,
            "wq": np.ascontiguousarray(wq_h[:, 2 * i:2 * i + 2]),
            "wk": np.ascontiguousarray(wk_h[:, kvh]),
            "wv": np.ascontiguousarray(wv_h[:, kvh]),
            "wo": wo_h,
            "wg": np.ascontiguousarray(
                _wlhsT_dr(tg[irow], IT).reshape(P, 4, 8, HT2, 2, P)
                .transpose(1, 0, 2, 3, 4, 5)),
            "wu": np.ascontiguousarray(
                _wlhsT_dr(tu[irow], IT).reshape(P, 4, 8, HT2, 2, P)
                .transpose(1, 0, 2, 3, 4, 5)),
            "wd": np.ascontiguousarray(_wd_layout(td[:, irow])[fo_perm]),
            "aq": np.ascontiguousarray(aq_h[:, 2 * i:2 * i + 2]),
            "ak": np.ascontiguousarray(ak_h[:, kvh:kvh + 1]),
            "av": np.ascontiguousarray(av_h[:, kvh:kvh + 1]),
            "ao": ao_h,
            "ag": np.ascontiguousarray(ag_h[:, isl]),
            "au": np.ascontiguousarray(au_h[:, isl]),
            "ad": np.ascontiguousarray(ad_h[:, fo_perm]),
            "rT": rT_h, "tril2": tril2_h, "dmka": dmka_h, "dmkb": dmkb_h,
            "iden": iden_h,
            "iden8": iden_h.astype(F8),
            "ones_f": ones_f, "ones_b": ones_b, "invh_b": invh_b,
            "ones2": ones2_h, "ones1": ones1_h, "invh2": invh2_h,
        })

    res = run_bass_kernel_spmd(nc, in_maps, list(range(NC)))
    _CACHE["last_result"] = res

    # ---- host-side unshard: xmid residual + pair-RS output assembly ----
    out_T = np.zeros((H, S), np.float64)
    for i in range(NC):
        blo, bhi = i, 15 - i
        xm = res.results[i]["xmidT"].astype(np.float64)      # [P, HT, 256]
        xm = xm.transpose(1, 0, 2).reshape(H, TOK)
        out_T[:, blo * P:(blo + 1) * P] += xm[:, 0:P]
        out_T[:, bhi * P:(bhi + 1) * P] += xm[:, P:TOK]
    for j in range(NC // 2):
        # pair token order: [core 2j's 256 | core 2j+1's 256]
        tok_cols = np.r_[(2 * j) * P:(2 * j + 1) * P,
                         (15 - 2 * j) * P:(16 - 2 * j) * P,
                         (2 * j + 1) * P:(2 * j + 2) * P,
                         (14 - 2 * j) * P:(15 - 2 * j) * P]
        for par in range(2):
            od = res.results[2 * j + par]["outD"].astype(np.float64)  # [8,2,128,512]
            od2 = res.results[2 * j + par]["outD2"].astype(np.float64)  # [8,128,512]
            for c in range(8):
                fo = 2 * c + par
                out_T[fo * P:(fo + 1) * P][:, tok_cols] += (
                    od[c, 0] + od[c, 1] + od2[c])
    return np.ascontiguousarray(out_T.T).reshape(1, S, H).astype(np.float32)


if __name__ == "__main__":
    nc = _build_program()
    print("build OK; instructions:",
          sum(len(b.instructions) for f in nc.m.functions for b in f.blocks))


# revision 51
# speedup vs baseline: 1.0319x; 1.0117x over previous
"""BitNet transformer block on 8 Trainium2 NeuronCores (Bass/Tile SPMD).

v3: same sharding as v2 (head-parallel attention with A2A to token-parallel,
pair TP-2 MLP), restructured for collective/compute overlap:
 - wo/xo/o_my prefetched during attention; o_proj runs as two token halves,
   half0 right after attention (A2A_lo already landed), half1 after A2A_hi.
 - ln2 + pair-AllGather chunked per token half, fired as each o_proj half
   completes; MLP gate/up weights chunk-0 prefetched during o_proj.
 - down-proj ReduceScatter in 8 chunks of 2 f-tiles; outD DMAs issued after
   the last RS trigger so the sync queue never blocks next-chunk weights.
 - engine rebalance: ln1 squares spread vector/gpsimd/scalar, MLP square on
   scalar so gpsimd stays trigger-only while collectives are in flight.
"""

import sys

import numpy as np

try:
    import concourse.bass as bass  # noqa: F401
except Exception:  # pragma: no cover
    sys.path.insert(0, "/opt/trn_rl_repo")

import ml_dtypes
import concourse.bass as bass
import concourse.mybir as mybir
import concourse.tile as tile
from concourse import bacc
from concourse.bass_utils import run_bass_kernel_spmd

FP32 = mybir.dt.float32
BF16 = mybir.dt.bfloat16
FP8 = mybir.dt.float8e4
BF = ml_dtypes.bfloat16
F8 = ml_dtypes.float8_e4m3

ALPHA = 0.7
EPS = 1e-5
NH = 16          # query heads
NKV = 4          # kv heads
D = 128          # head dim
H = 2048         # hidden
I_TOT = 8192     # mlp intermediate
S = 2048         # sequence
NC = 8           # cores
P = 128
HT = H // P      # 16 hidden tiles
HT2 = HT // 2    # 8 hidden tile-pairs (fp8 DoubleRow)
B = S // P       # 16 token blocks
I_LOC = I_TOT // 2    # 4096 intermediate per core (TP-2)
IT = I_LOC // P       # 32 inter tiles per core
TOK = 256             # tokens owned per core (2 blocks)
PTOK = 512            # tokens owned per pair
DR = mybir.MatmulPerfMode.DoubleRow

_CACHE = {}


def _build_program():
    nc = bacc.Bacc("TRN2", target_bir_lowering=False, debug=False, num_devices=NC)
    AF = mybir.ActivationFunctionType
    ALU = mybir.AluOpType
    rg_all = [list(range(NC))]
    rg_pair = [[2 * j, 2 * j + 1] for j in range(NC // 2)]

    # ---------------- inputs ----------------
    def dram_in(name, shape, dt=FP32):
        return nc.dram_tensor(name, shape, dt, kind="ExternalInput")

    xT_f = dram_in("xT_f", [P, HT, S], FP8)           # fp8 x^T ALL tokens (ln1 only)
    xT_own = dram_in("xT_own", [P, HT, TOK])          # fp32 x^T own cols (residual)
    cos_f = dram_in("cos_f", [P, S], BF16)
    sin_f = dram_in("sin_f", [P, S], BF16)
    wq_in = dram_in("wq", [P, 2, HT2, 2, P], FP8)     # my 2 heads [p,f,b,i,m]
    wk_in = dram_in("wk", [P, HT2, 2, P], FP8)        # my kv head
    wv_in = dram_in("wv", [P, HT2, 2, P], FP8)
    wo_in = dram_in("wo", [HT, P, HT, P], FP8)
    wg_in = dram_in("wg", [IT // 8, P, 8, HT2, 2, P], FP8)  # [g,p,f8,b,i,m]
    wu_in = dram_in("wu", [IT // 8, P, 8, HT2, 2, P], FP8)
    wd_in = dram_in("wd", [HT, P, IT // 2, 2, P], FP8)  # [fo,p,b,i,m]
    aq_in = dram_in("aq", [P, 2])
    ak_in = dram_in("ak", [P, 1])
    av_in = dram_in("av", [P, 1])
    ao_in = dram_in("ao", [P, HT])
    ag_in = dram_in("ag", [P, IT])
    au_in = dram_in("au", [P, IT])
    ad_in = dram_in("ad", [P, HT])
    rT_in = dram_in("rT", [P, P], BF16)               # rope rotate-half perm^T
    tril_in = dram_in("tril2", [P, TOK], BF16)        # [k, q] keep k<=q, both heads
    dmka_in = dram_in("dmka", [P, 4 * P], BF16)       # diag kb=2m: [tri|1|tri|1]
    dmkb_in = dram_in("dmkb", [P, 4 * P], BF16)       # diag kb=2m+1: [0|tri|0|tri]
    iden_in = dram_in("iden", [P, P], BF16)           # identity for PE transpose
    iden8_in = dram_in("iden8", [P, P], FP8)          # fp8 identity
    ones_f_in = dram_in("ones_f", [P, P])             # fp32 ones
    ones_b_in = dram_in("ones_b", [P, 1], BF16)       # bf16 ones column
    ones2_in = dram_in("ones2", [P, 2, 16], FP8)      # fp8 ones (DR lps lhsT)
    ones1_in = dram_in("ones1", [P, 16], FP8)         # fp8 ones (lps lhsT)
    invh_b_in = dram_in("invh_b", [P, 1], BF16)       # bf16 1/H column
    invh2_in = dram_in("invh2", [P, 2, 16], FP8)      # fp8 2^-9 (DR ssq lhsT)

    xmidT = nc.dram_tensor("xmidT", [P, HT, TOK], FP32, kind="ExternalOutput")
    outD = nc.dram_tensor("outD", [8, 2, P, PTOK], BF16, kind="ExternalOutput")
    outD2 = nc.dram_tensor("outD2", [8, P, PTOK], BF16, kind="ExternalOutput")

    a2a_lo_in = nc.dram_tensor("a2a_lo_in", [NC, P, 2, P], FP8)
    a2a_lo_out = nc.dram_tensor("a2a_lo_out", [NC, P, 2, P], FP8)
    a2a_hi_in = nc.dram_tensor("a2a_hi_in", [NC, P, 2, P], FP8)
    a2a_hi_out = nc.dram_tensor("a2a_hi_out", [NC, P, 2, P], FP8)
    # pair exchanges ride 2-rank AllGather (faster than 2-rank RS per byte).
    # down-sum, minimal wire: per-core wd/ad fo order is [mine, partner's]
    # (host permutes in_maps), slot-1 partial (partner's fo) is gathered;
    # final = (out[0]+out[1]) - sent + kept — sent cancels bit-exact, so the
    # program never needs to know its own pair parity.
    agx_in = [nc.dram_tensor(f"agx_in_{h}", [P, HT, P], FP8) for h in range(2)]
    agx_out = [nc.dram_tensor(f"agx_out_{h}", [2, P, HT, P], FP8)
               for h in range(2)]
    rs_in = [nc.dram_tensor(f"rs_in_{c}", [P, PTOK], BF16) for c in range(8)]
    rs_out = [nc.dram_tensor(f"rs_out_{c}", [2, P, PTOK], BF16)
              for c in range(8)]

    with tile.TileContext(nc) as tc:
        const = tc.alloc_tile_pool(name="const", bufs=1)
        ones_f = const.tile([P, P], FP32)
        ones_b = const.tile([P, 1], BF16)
        ones2 = const.tile([P, 2, 16], FP8)
        ones1 = const.tile([P, 16], FP8)
        invh_b = const.tile([P, 1], BF16)
        invh2 = const.tile([P, 2, 16], FP8)
        rT = const.tile([P, P], BF16)
        iden = const.tile([P, P], BF16)
        iden8 = const.tile([P, P], FP8)
        tril2 = const.tile([P, TOK], BF16)
        dmka = const.tile([P, 4 * P], BF16)
        dmkb = const.tile([P, 4 * P], BF16)
        aq = const.tile([P, 2], FP32)
        ak = const.tile([P, 1], FP32)
        av = const.tile([P, 1], FP32)
        ao = const.tile([P, HT], FP32)
        ag = const.tile([P, IT], FP32)
        au = const.tile([P, IT], FP32)
        ad = const.tile([P, HT], FP32)
        # order: tiny tiles needed by the first ssq/projection chain first;
        # late-use scale tiles (aq..ad) last so they don't delay chunk 0.
        for dst, src in [(invh2, invh2_in), (ones_f, ones_f_in), (rT, rT_in),
                         (iden, iden_in), (ones2, ones2_in), (ones1, ones1_in),
                         (ones_b, ones_b_in), (invh_b, invh_b_in),
                         (iden8, iden8_in), (tril2, tril_in),
                         (dmka, dmka_in), (dmkb, dmkb_in),
                         (aq, aq_in), (ak, ak_in), (av, av_in), (ao, ao_in),
                         (ag, ag_in), (au, au_in), (ad, ad_in)]:
            nc.sync.dma_start(dst[:], src[:])

        midpool = tc.alloc_tile_pool(name="midpool", bufs=1)
        x_mid = midpool.tile([P, HT, TOK], FP32)
        xopool = tc.alloc_tile_pool(name="xopool", bufs=1)
        xo = xopool.tile([P, HT, TOK], FP32)
        omypool = tc.alloc_tile_pool(name="omypool", bufs=1)
        o_my = omypool.tile([P, HT, TOK], FP8)      # post-A2A: 16 heads x my toks
        wores = tc.alloc_tile_pool(name="wo_res", bufs=1)
        wo_all = wores.tile([P, HT, HT, P], FP8)
        qkvpool = tc.alloc_tile_pool(name="qkvpool", bufs=1)
        q_my = qkvpool.tile([P, 2, S], BF16)         # my 2 heads, all tokens
        k_my = qkvpool.tile([P, B, P], BF16)         # my kv head [d, blk, tok]
        v_my = qkvpool.tile([P, B, P], FP8)          # my kv head [tok, blk, d]

        # ====== phase 1: ln1 (all tokens, chunked) + q/k/v TP projections ======
        CH4 = 512
        with tc.tile_pool(name="xc_pool", bufs=2) as xcp, \
             tc.tile_pool(name="p1sb", bufs=2) as p1sb, \
             tc.tile_pool(name="p1ps", bufs=1, space="PSUM") as p1ps, \
             tc.tile_pool(name="p2ps", bufs=2, space="PSUM") as p2ps, \
             tc.tile_pool(name="rot_ps", bufs=2, space="PSUM") as rot_ps, \
             tc.tile_pool(name="vt_ps", bufs=2, space="PSUM") as vt_ps, \
             tc.tile_pool(name="p2sb", bufs=2) as p2sb, \
             tc.tile_pool(name="cs_pool", bufs=2) as csp, \
             tc.tile_pool(name="wres", bufs=1) as wres:
            # chunk-0 x lands before the projection weights: the ssq square
            # chain (vector/gpsimd/scalar) only needs x, weights are needed
            # a few microseconds later at the first q matmul.
            wq_sb = wres.tile([P, 2, HT2, 2, P], FP8)
            wk_sb = wres.tile([P, HT2, 2, P], FP8)
            wv_sb = wres.tile([P, HT2, 2, P], FP8)
            for c4 in range(4):
                tsl = slice(c4 * CH4, (c4 + 1) * CH4)
                xc = xcp.tile([P, HT, CH4], FP8, name="xc")
                # split the 1MB chunk across both DMA queues so the first
                # hidden tiles land sooner (the ssq chain consumes in order)
                nc.scalar.dma_start(xc[:, 0:HT // 2, :], xT_f[:, 0:HT // 2, tsl])
                nc.sync.dma_start(xc[:, HT // 2:, :], xT_f[:, HT // 2:, tsl])
                if c4 == 0:
                    nc.scalar.dma_start(wq_sb[:], wq_in[:])
                    nc.scalar.dma_start(wk_sb[:], wk_in[:])
                    nc.scalar.dma_start(wv_sb[:], wv_in[:])
                cfc = csp.tile([P, CH4], BF16, name="cfc")
                nc.sync.dma_start(cfc[:], cos_f[:, tsl])
                sfc = csp.tile([P, CH4], BF16, name="sfc")
                nc.sync.dma_start(sfc[:], sin_f[:, tsl])
                # rmsnorm scale from fp8 x; applied per-token at PSUM evict.
                # squares spread across vector/gpsimd/scalar (DVE was the
                # phase-1 co-bottleneck).
                ssq = p1ps.tile([16, CH4], FP32, name="ssq")
                for b in range(HT2):
                    sq2 = p1sb.tile([P, 2, CH4], FP8, name="sq2")
                    for j in range(2):
                        k = (2 * b + j) % 3
                        if k == 0:
                            nc.vector.tensor_mul(sq2[:, j, :], xc[:, 2 * b + j, :],
                                                 xc[:, 2 * b + j, :])
                        elif k == 1:
                            nc.gpsimd.tensor_mul(sq2[:, j, :], xc[:, 2 * b + j, :],
                                                 xc[:, 2 * b + j, :])
                        else:
                            nc.scalar.activation(sq2[:, j, :], xc[:, 2 * b + j, :],
                                                 AF.Square)
                    nc.tensor.matmul(ssq[:], invh2[:], sq2[:],
                                     start=(b == 0), stop=(b == HT2 - 1),
                                     perf_mode=DR)
                ssb = p1sb.tile([1, CH4], FP32, name="ssb")
                nc.scalar.activation(ssb[:], ssq[0:1, :], AF.Copy)
                msb = p1ps.tile([P, CH4], FP32, name="msb")
                nc.tensor.matmul(msb[:], ones_f[0:1, :], ssb[:],
                                 start=True, stop=True)
                rec = p1sb.tile([P, CH4], FP32, name="rec")
                nc.vector.reciprocal_approx_fast(rec[:], msb[:])
                rsq_bc = p1sb.tile([P, CH4], FP32, name="rsq_bc", tag="rsq_bc")
                nc.scalar.activation(rsq_bc[:], rec[:], AF.Sqrt, scale=4.0)
                # q (2 heads): both PSUM chains first, then both rotations,
                # then the elementwise rope stage — avoids the per-head
                # PE->vector->PE ping-pong (head-of-line stalls on both).
                qs2 = [None, None]
                for f in range(2):
                    ps = p2ps.tile([P, CH4], FP32, name="pps")
                    for b in range(HT2):
                        nc.tensor.matmul(ps[:], wq_sb[:, f, b, :, :],
                                         xc[:, 2 * b:2 * b + 2, :],
                                         start=(b == 0), stop=(b == HT2 - 1),
                                         perf_mode=DR)
                    qs = p2sb.tile([P, CH4], BF16, name=f"qs{f}")
                    nc.vector.scalar_tensor_tensor(qs[:], ps[:], aq[:, f:f + 1],
                                                   rsq_bc[:], ALU.mult, ALU.mult)
                    qs2[f] = qs
                rot2 = [None, None]
                for f in range(2):
                    rot = rot_ps.tile([P, CH4], FP32, name="rot")
                    nc.tensor.matmul(rot[:], rT[:], qs2[f][:],
                                     start=True, stop=True)
                    rot2[f] = rot
                for f in range(2):
                    t2 = p2sb.tile([P, CH4], BF16, name="t2")
                    nc.vector.tensor_mul(t2[:], qs2[f][:], cfc[:])
                    t1 = p2sb.tile([P, CH4], BF16, name="t1")
                    nc.vector.tensor_mul(t1[:], rot2[f][:], sfc[:])
                    nc.gpsimd.tensor_add(q_my[:, f, tsl], t1[:], t2[:])
                # k: my kv head
                ps = p2ps.tile([P, CH4], FP32, name="pps")
                for b in range(HT2):
                    nc.tensor.matmul(ps[:], wk_sb[:, b, :, :],
                                     xc[:, 2 * b:2 * b + 2, :],
                                     start=(b == 0), stop=(b == HT2 - 1),
                                     perf_mode=DR)
                ks = p2sb.tile([P, CH4], BF16, name="qs0")
                nc.vector.scalar_tensor_tensor(ks[:], ps[:], ak[:, 0:1],
                                               rsq_bc[:], ALU.mult, ALU.mult)
                rot = rot_ps.tile([P, CH4], FP32, name="rot")
                nc.tensor.matmul(rot[:], rT[:], ks[:], start=True, stop=True)
                t2 = p2sb.tile([P, CH4], BF16, name="t2")
                nc.vector.tensor_mul(t2[:], ks[:], cfc[:])
                t1 = p2sb.tile([P, CH4], BF16, name="t1")
                nc.vector.tensor_mul(t1[:], rot[:], sfc[:])
                nc.gpsimd.tensor_add(
                    k_my[:, 4 * c4:4 * c4 + 4, :].rearrange("p b t -> p (b t)"),
                    t1[:], t2[:])
                # v: my kv head, then PE-transpose to [tok, d] (fp8)
                ps = p2ps.tile([P, CH4], FP32, name="pps")
                for b in range(HT2):
                    nc.tensor.matmul(ps[:], wv_sb[:, b, :, :],
                                     xc[:, 2 * b:2 * b + 2, :],
                                     start=(b == 0), stop=(b == HT2 - 1),
                                     perf_mode=DR)
                vtv = p2sb.tile([P, CH4], BF16, name="vtv")
                nc.vector.scalar_tensor_tensor(vtv[:], ps[:], av[:, 0:1],
                                               rsq_bc[:], ALU.mult, ALU.mult)
                for j in range(4):
                    vtp = vt_ps.tile([P, P], BF16, name="vtp")
                    nc.tensor.transpose(vtp[:], vtv[:, j * P:(j + 1) * P], iden[:])
                    nc.vector.tensor_copy(v_my[:, 4 * c4 + j, :], vtp[:])

        # ========= prefetch for phase 3 (overlaps attention on DMA) =========
        for g in range(4):
            eng = nc.sync if g % 2 == 0 else nc.scalar
            eng.dma_start(wo_all[:, 4 * g:4 * (g + 1), :, :],
                          wo_in[4 * g:4 * (g + 1)].rearrange("f p k m -> p f k m"))
        nc.sync.dma_start(xo[:], xT_own[:])

        # ====== phase 2: attention (two consecutive query blocks fused) ======
        # q blocks {2m, 2m+1} processed together: FD=512 matmuls (columns
        # [h, qb, t]); the diagonal kb pair uses mask constants
        # dmka = [tri|1|tri|1] (kb=2m), dmkb = [0|tri|0|tri] (kb=2m+1).
        with tc.tile_pool(name="a_ps", bufs=2, space="PSUM") as a_ps, \
             tc.tile_pool(name="o_ps", bufs=2, space="PSUM") as o_ps, \
             tc.tile_pool(name="lbc_ps", bufs=1, space="PSUM") as lbc_ps, \
             tc.tile_pool(name="a_sb", bufs=3) as a_sb:
            for m in range(8):
                qb0 = 2 * m
                q_pair = q_my[:, :, qb0 * P:(qb0 + 2) * P]   # [P, 2, 256]
                ops = o_ps.tile([P, 2, 2, P], FP32, name="ops")   # [d][h][q][t]
                lps = lbc_ps.tile([16, 4 * P], FP32, name="lps")
                for g in range(m + 1):
                    kb0 = 2 * g
                    first, diag = g == 0, g == m
                    sps = a_ps.tile([P, 2, 4 * P], FP32, name="sps")
                    for j in range(2):
                        nc.tensor.matmul(sps[:, j, :], k_my[:, kb0 + j, :],
                                         q_pair, start=True, stop=True)
                    pm2 = a_sb.tile([P, 2, 4 * P], FP8, name="pm2")
                    if diag:
                        pmd = a_sb.tile([P, 2, 4 * P], BF16, name="pmd")
                        nc.scalar.activation(
                            pmd[:].rearrange("p a t -> p (a t)"),
                            sps[:].rearrange("p a t -> p (a t)"), AF.Exp)
                        nc.vector.tensor_mul(pm2[:, 0, :], pmd[:, 0, :], dmka[:])
                        nc.vector.tensor_mul(pm2[:, 1, :], pmd[:, 1, :], dmkb[:])
                    else:
                        nc.scalar.activation(
                            pm2[:].rearrange("p a t -> p (a t)"),
                            sps[:].rearrange("p a t -> p (a t)"), AF.Exp)
                    nc.tensor.matmul(lps[:], ones2[:], pm2[:],
                                     start=first, stop=diag, perf_mode=DR)
                    nc.tensor.matmul(ops[:].rearrange("p h q t -> p (h q t)"),
                                     v_my[:, kb0:kb0 + 2, :], pm2[:],
                                     start=first, stop=diag, perf_mode=DR)
                lsb = a_sb.tile([1, 4 * P], FP32, name="lsb")
                nc.scalar.activation(lsb[:], lps[0:1, :], AF.Copy)
                bca = lbc_ps.tile([P, 4 * P], FP32, name="bca")
                nc.tensor.matmul(bca[:], ones_f[0:1, :], lsb[:],
                                 start=True, stop=True)
                linv = a_sb.tile([P, 4 * P], FP32, name="linv")
                nc.vector.reciprocal_approx_fast(linv[:], bca[:])
                osb = a_sb.tile([P, 2, 2, P], FP8, name="osb")
                nc.vector.tensor_mul(
                    osb[:].rearrange("p h q t -> p (h q t)"),
                    ops[:].rearrange("p h q t -> p (h q t)"), linv[:])
                for j in range(2):
                    qb = qb0 + j
                    r_dst = min(qb, 15 - qb)
                    dst = a2a_lo_in if qb < 8 else a2a_hi_in
                    nc.sync.dma_start(dst[r_dst][:], osb[:, :, j, :])
                if m == 3:
                    nc.gpsimd.collective_compute(
                        "AllToAll", ALU.bypass, ins=[a2a_lo_in[:]],
                        outs=[a2a_lo_out[:]], replica_groups=rg_all)
                if m == 5:
                    # A2A_lo has landed by now; pull the low-half heads in
                    # while the tail query blocks compute.
                    for j in range(NC):
                        nc.sync.dma_start(o_my[:, 2 * j:2 * j + 2, 0:P],
                                          a2a_lo_out[j])
            nc.gpsimd.collective_compute(
                "AllToAll", ALU.bypass, ins=[a2a_hi_in[:]],
                outs=[a2a_hi_out[:]], replica_groups=rg_all)
        qkvpool.release()

        # ===== phase 3: o_proj + residual + ln2 (token halves) + pair-AG =====
        # MLP pools allocated early so gate/up chunk 0 prefetches during o_proj
        h2cp = tc.alloc_tile_pool(name="h2c_pool", bufs=1)
        h2c = h2cp.tile([P, HT, PTOK], FP8)
        mp = tc.alloc_tile_pool(name="m_pool", bufs=1)
        m_all = mp.tile([P, IT, PTOK], FP8)
        wgup = tc.alloc_tile_pool(name="wgu_pool", bufs=2)
        wtg = [None] * 4
        wtu = [None] * 4
        wtg[0] = wgup.tile([P, 8, HT2, 2, P], FP8, name="wtg8")
        nc.sync.dma_start(wtg[0][:], wg_in[0])
        wtu[0] = wgup.tile([P, 8, HT2, 2, P], FP8, name="wtu8")
        nc.scalar.dma_start(wtu[0][:], wu_in[0])

        with tc.tile_pool(name="p5ps", bufs=2, space="PSUM") as p5ps, \
             tc.tile_pool(name="p5sb", bufs=3) as p5sb, \
             tc.tile_pool(name="h2h_pool", bufs=1) as h2hp:
            h2h = h2hp.tile([P, HT, TOK], FP8)
            # high-half heads: sync queue waits on A2A_hi while PE runs half 0
            for j in range(NC):
                nc.sync.dma_start(o_my[:, 2 * j:2 * j + 2, P:TOK],
                                  a2a_hi_out[j])
            ssq2 = p5ps.tile([16, TOK], FP32, name="ssq2")
            for half in range(2):
                csl = slice(half * P, (half + 1) * P)
                # ln2 sum-of-squares interleaved into the o_proj f-loop at
                # pair granularity so the PE never drains waiting on the
                # DVE/ACT square chain after the last f-tile.
                for f in range(HT):
                    ps = p5ps.tile([P, P], FP32, name="ops5")
                    for kt in range(HT):
                        nc.tensor.matmul(ps[:], wo_all[:, f, kt, :], o_my[:, kt, csl],
                                         start=(kt == 0), stop=(kt == HT - 1))
                    nc.vector.scalar_tensor_tensor(
                        x_mid[:, f, csl], ps[:], ao[:, f:f + 1],
                        xo[:, f, csl], ALU.mult, ALU.add)
                    if f % 2 == 1:
                        b = f // 2
                        sq2 = p5sb.tile([P, 2, P], FP8, name="sq2h")
                        nc.vector.tensor_mul(sq2[:, 0, :],
                                             x_mid[:, 2 * b, csl],
                                             x_mid[:, 2 * b, csl])
                        nc.scalar.activation(sq2[:, 1, :],
                                             x_mid[:, 2 * b + 1, csl],
                                             AF.Square)
                        nc.tensor.matmul(ssq2[:, csl], invh2[:], sq2[:],
                                         start=(b == 0), stop=(b == HT2 - 1),
                                         perf_mode=DR)
                ssb2 = p5sb.tile([1, P], FP32, name="ssb2")
                nc.scalar.activation(ssb2[:], ssq2[0:1, csl], AF.Copy)
                msb2 = p5ps.tile([P, P], FP32, name="msb2")
                nc.tensor.matmul(msb2[:], ones_f[0:1, :], ssb2[:],
                                 start=True, stop=True)
                rec2 = p5sb.tile([P, P], FP32, name="rec2")
                nc.vector.reciprocal_approx_fast(rec2[:], msb2[:])
                rsq2 = p5sb.tile([P, P], FP32, name="rsq2")
                nc.scalar.activation(rsq2[:], rec2[:], AF.Sqrt, scale=4.0)
                for kt in range(HT):
                    # gpsimd only before its first trigger (AG0) is enqueued:
                    # a trigger may block the queue until the CC completes.
                    eng = (nc.gpsimd if (half == 0 and kt % 2 == 1)
                           else nc.vector)
                    eng.tensor_mul(h2h[:, kt, csl], x_mid[:, kt, csl],
                                   rsq2[:])
                nc.sync.dma_start(agx_in[half][:], h2h[:, :, csl])
                nc.gpsimd.collective_compute(
                    "AllGather", ALU.bypass, ins=[agx_in[half][:]],
                    outs=[agx_out[half][:]], replica_groups=rg_pair)
                nc.scalar.dma_start(xmidT[:, :, csl], x_mid[:, :, csl])

        # ========== phase 4: MLP (pair TP-2 over inter) + chunked RS ==========
        with tc.tile_pool(name="wd_pool", bufs=4) as wdp, \
             tc.tile_pool(name="p7ps", bufs=2, space="PSUM") as p7ps, \
             tc.tile_pool(name="p7dps", bufs=2, space="PSUM") as p7dps, \
             tc.tile_pool(name="p7sb", bufs=4) as p7sb, \
             tc.tile_pool(name="rsum", bufs=2) as rsum:
            # pair token order: [2j's 256 | (2j+1)'s 256], each = [lo128|hi128]
            for half in range(2):
                for r in range(2):
                    eng = nc.sync if r == 0 else nc.scalar
                    eng.dma_start(
                        h2c[:, :, r * TOK + half * P:r * TOK + (half + 1) * P],
                        agx_out[half][r])
            for f in range(IT):
                if f % 8 == 0 and f // 8 + 1 < 4:
                    k = f // 8 + 1
                    wtg[k] = wgup.tile([P, 8, HT2, 2, P], FP8, name="wtg8")
                    nc.sync.dma_start(wtg[k][:], wg_in[k])
                    wtu[k] = wgup.tile([P, 8, HT2, 2, P], FP8, name="wtu8")
                    nc.scalar.dma_start(wtu[k][:], wu_in[k])
                wtg8, wtu8 = wtg[f // 8], wtu[f // 8]
                gps = p7ps.tile([P, PTOK], FP32, name="gps")
                for b in range(HT2):
                    nc.tensor.matmul(gps[:], wtg8[:, f % 8, b, :, :],
                                     h2c[:, 2 * b:2 * b + 2, :],
                                     start=(b == 0), stop=(b == HT2 - 1),
                                     perf_mode=DR)
                ups = p7ps.tile([P, PTOK], FP32, name="ups")
                for b in range(HT2):
                    nc.tensor.matmul(ups[:], wtu8[:, f % 8, b, :, :],
                                     h2c[:, 2 * b:2 * b + 2, :],
                                     start=(b == 0), stop=(b == HT2 - 1),
                                     perf_mode=DR)
                gr = p7sb.tile([P, PTOK], BF16, name="gr")
                nc.vector.tensor_scalar(gr[:], gps[:], ag[:, f:f + 1], 0.0,
                                        ALU.mult, ALU.max)
                g2 = p7sb.tile([P, PTOK], BF16, name="g2")
                nc.scalar.activation(g2[:], gr[:], AF.Square)
                nc.vector.scalar_tensor_tensor(m_all[:, f, :], ups[:],
                                               au[:, f:f + 1], g2[:],
                                               ALU.mult, ALU.mult)
            # down proj in 8 chunks of 2 fo; slot 0 = MY fo (kept local),
            # slot 1 = partner's fo (gathered). dd = kept - sent goes out via
            # outD2 off the critical path; the gathered slots go out raw via
            # a single dram->dram DMA per chunk (host does the 3-way add in
            # f64: fin = out[0] + out[1] + dd — the sent tile cancels).
            def consume_rs(c):
                eng = nc.sync if c % 2 == 0 else nc.scalar
                eng.dma_start(outD[c], rs_out[c][:])

            for c in range(8):
                dn2 = [None, None]
                for j in range(2):
                    fo = 2 * c + j
                    wtd = wdp.tile([P, IT // 2, 2, P], FP8, name="wtd")
                    nc.sync.dma_start(wtd[:], wd_in[fo])
                    dps = p7dps.tile([P, PTOK], FP32, name="dps")
                    for b in range(IT // 2):
                        nc.tensor.matmul(dps[:], wtd[:, b, :, :],
                                         m_all[:, 2 * b:2 * b + 2, :],
                                         start=(b == 0), stop=(b == IT // 2 - 1),
                                         perf_mode=DR)
                    dn = p7sb.tile([P, PTOK], BF16, name=f"dn{j}")
                    nc.scalar.activation(dn[:], dps[:], AF.Copy,
                                         scale=ad[:, fo:fo + 1])
                    dn2[j] = dn
                nc.sync.dma_start(rs_in[c][:], dn2[1][:])
                dd = p7sb.tile([P, PTOK], BF16, name="dd")
                nc.vector.tensor_sub(dd[:], dn2[0][:], dn2[1][:])
                nc.scalar.dma_start(outD2[c], dd[:])
                nc.gpsimd.collective_compute(
                    "AllGather", ALU.bypass, ins=[rs_in[c][:]],
                    outs=[rs_out[c][:]], replica_groups=rg_pair)
                if c >= 2:
                    consume_rs(c - 2)
            consume_rs(6)
            consume_rs(7)
        wgup.release()
        mp.release()
        h2cp.release()
        wores.release()
        omypool.release()
        xopool.release()
        midpool.release()
        const.release()

    nc.finalize()
    return nc


def _ternary(w, fold_row=None):
    """Quantize [O, Hin] fp32 -> (ternary fp32 {-1,0,1}, absmean [O])."""
    w = np.asarray(w, dtype=np.float32)
    am = np.mean(np.abs(w), axis=1)
    t = np.sign(w) * (np.abs(w) > ALPHA * am[:, None]).astype(np.float32)
    if fold_row is not None:
        t = t * fold_row[None, :]
    return t, am


def _wlhsT(tern, n_f):
    """ternary [O, Hin] -> bf16 lhsT layout [f, p, kt, c]."""
    o, hin = tern.shape
    kt = hin // P
    assert n_f * P == o
    wT = np.ascontiguousarray(tern.T)  # [Hin, O]
    return np.ascontiguousarray(
        wT.reshape(kt, P, n_f, P).transpose(2, 1, 0, 3)).astype(BF)


def _wlhsT_dr(tern, n_f):
    """ternary [O, Hin] -> fp8 DoubleRow lhsT layout [p, f, b, i, m]:
    w[p, f, b, i, m] = ternT[128*(2b+i)+p, 128*f+m]."""
    o, hin = tern.shape
    b2 = hin // (2 * P)
    assert n_f * P == o
    wT = np.ascontiguousarray(tern.T)  # [Hin, O]
    return np.ascontiguousarray(
        wT.reshape(b2, 2, P, n_f, P).transpose(2, 3, 0, 1, 4)).astype(F8)


def _wd_layout(td_slice):
    """[H, I_loc] -> fp8 DoubleRow [fo, p, b, i, m]: wd[fo, p, b, i, m] =
    td_slice[128*fo+m, 128*(2b+i)+p]."""
    hin, iloc = td_slice.shape
    assert hin == H and iloc == I_LOC
    wT = np.ascontiguousarray(td_slice.T)  # [I_loc, H]
    return np.ascontiguousarray(
        wT.reshape(IT // 2, 2, P, HT, P).transpose(3, 2, 0, 1, 4)).astype(F8)


def _scale_tiles(a):
    """[O] -> [P, O//P] with column f = features f*128..f*128+127."""
    return np.ascontiguousarray(a.reshape(-1, P).T).astype(np.float32)


def _pcol(x2d):
    """[K, T] -> [P, K//P, T] (partition-major for direct DMA)."""
    k, t = x2d.shape
    return np.ascontiguousarray(
        x2d.reshape(k // P, P, t).transpose(1, 0, 2)).astype(np.float32)


def kernel(x, cos, sin, wq, wk, wv, wo, wg, wu, wd, ln1_w, ln2_w):
    x = np.asarray(x, dtype=np.float32)
    b, s, hdim = x.shape
    assert (b, s, hdim) == (1, S, H)

    if "nc" not in _CACHE:
        _CACHE["nc"] = _build_program()
    nc = _CACHE["nc"]

    ln1 = np.asarray(ln1_w, dtype=np.float32)
    ln2 = np.asarray(ln2_w, dtype=np.float32)

    tq, amq = _ternary(wq, fold_row=ln1)
    tk, amk = _ternary(wk, fold_row=ln1)
    tv, amv = _ternary(wv, fold_row=ln1)
    to, amo = _ternary(wo)
    tg, amg = _ternary(wg, fold_row=ln2)
    tu, amu = _ternary(wu, fold_row=ln2)
    td, amd = _ternary(wd)

    wq_h = _wlhsT_dr(tq, NH)          # [P, 16, 8, 2, P]
    wk_h = _wlhsT_dr(tk, NKV)         # [P, 4, 8, 2, P]
    wv_h = _wlhsT_dr(tv, NKV)
    wo_h = _wlhsT(to, HT).astype(F8)  # [16, P, 16, P] fp8

    aq_h = _scale_tiles(amq / np.sqrt(np.float32(D)))
    ak_h = _scale_tiles(amk)
    av_h = _scale_tiles(amv)
    ao_h = _scale_tiles(amo)
    ag_h = _scale_tiles(amg)          # [P, 64]
    au_h = _scale_tiles(amu)
    ad_h = _scale_tiles(amd)          # [P, 16]

    x2 = x[0]
    xT = np.ascontiguousarray(x2.T)
    xT_f = _pcol(xT)
    cosT = np.ascontiguousarray(np.asarray(cos, np.float32)[0, 0].T).astype(BF)
    sinT = np.ascontiguousarray(np.asarray(sin, np.float32)[0, 0].T).astype(BF)

    R = np.zeros((P, P), np.float32)
    for m in range(64):
        R[m, m + 64] = -1.0
        R[m + 64, m] = 1.0
    rT_h = np.ascontiguousarray(R.T).astype(BF)
    ones_f = np.ones((P, P), np.float32)
    ones_b = np.ones((P, 1), np.float32).astype(BF)
    invh_b = np.full((P, 1), 1.0 / H, np.float32).astype(BF)
    ones2_h = np.ones((P, 2, 16), np.float32).astype(F8)
    invh2_h = np.full((P, 2, 16), 2.0 ** -9, np.float32).astype(F8)
    ones1_h = np.ones((P, 16), np.float32).astype(F8)
    triu = np.triu(np.ones((P, P), np.float32))
    tril2_h = np.ascontiguousarray(np.concatenate([triu, triu], axis=1)).astype(BF)
    onep = np.ones((P, P), np.float32)
    zep = np.zeros((P, P), np.float32)
    dmka_h = np.ascontiguousarray(
        np.concatenate([triu, onep, triu, onep], axis=1)).astype(BF)
    dmkb_h = np.ascontiguousarray(
        np.concatenate([zep, triu, zep, triu], axis=1)).astype(BF)
    iden_h = np.eye(P, dtype=np.float32).astype(BF)

    in_maps = []
    for i in range(NC):
        blo, bhi = i, 15 - i
        own_cols = np.r_[blo * P:(blo + 1) * P, bhi * P:(bhi + 1) * P]
        kvh = i // 2
        par = i % 2
        isl = slice(par * IT, (par + 1) * IT)       # inter tile slice (TP-2)
        irow = slice(par * I_LOC, (par + 1) * I_LOC)
        # down-proj fo slot order per core: [mine (2c+par), partner's]
        fo_perm = [2 * c + (j ^ par) for c in range(8) for j in range(2)]
        in_maps.append({
            "xT_f": xT_f.astype(F8),
            "xT_own": _pcol(xT[:, own_cols]),
            "cos_f": cosT, "sin_f": sinT,
            "wq": np.ascontiguousarray(wq_h[:, 2 * i:2 * i + 2]),
            "wk": np.ascontiguousarray(wk_h[:, kvh]),
            "wv": np.ascontiguousarray(wv_h[:, kvh]),
            "wo": wo_h,
            "wg": np.ascontiguousarray(
                _wlhsT_dr(tg[irow], IT).reshape(P, 4, 8, HT2, 2, P)
                .transpose(1, 0, 2, 3, 4, 5)),
            "wu": np.ascontiguousarray(
                _wlhsT_dr(tu[irow], IT).reshape(P, 4, 8, HT2, 2, P)
                .transpose(1, 0, 2, 3, 4, 5)),
            "wd": np.ascontiguousarray(_wd_layout(td[:, irow])[fo_perm]),
            "aq": np.ascontiguousarray(aq_h[:, 2 * i:2 * i + 2]),
            "ak": np.ascontiguousarray(ak_h[:, kvh:kvh + 1]),
            "av": np.ascontiguousarray(av_h[:, kvh:kvh + 1]),
            "ao": ao_h,
            "ag": np.ascontiguousarray(ag_h[:, isl]),
            "au": np.ascontiguousarray(au_h[:, isl]),
            "ad": np.ascontiguousarray(ad_h[:, fo_perm]),
            "rT": rT_h, "tril2": tril2_h, "dmka": dmka_h, "dmkb": dmkb_h,
            "iden": iden_h,
            "iden8": iden_h.astype(F8),
            "ones_f": ones_f, "ones_b": ones_b, "invh_b": invh_b,
            "ones2": ones2_h, "ones1": ones1_h, "invh2": invh2_h,
        })

    res = run_bass_kernel_spmd(nc, in_maps, list(range(NC)))
    _CACHE["last_result"] = res

    # ---- host-side unshard: xmid residual + pair-RS output assembly ----
    out_T = np.zeros((H, S), np.float64)
    for i in range(NC):
        blo, bhi = i, 15 - i
        xm = res.results[i]["xmidT"].astype(np.float64)      # [P, HT, 256]
        xm = xm.transpose(1, 0, 2).reshape(H, TOK)
        out_T[:, blo * P:(blo + 1) * P] += xm[:, 0:P]
        out_T[:, bhi * P:(bhi + 1) * P] += xm[:, P:TOK]
    for j in range(NC // 2):
        # pair token order: [core 2j's 256 | core 2j+1's 256]
        tok_cols = np.r_[(2 * j) * P:(2 * j + 1) * P,
                         (15 - 2 * j) * P:(16 - 2 * j) * P,
                         (2 * j + 1) * P:(2 * j + 2) * P,
                         (14 - 2 * j) * P:(15 - 2 * j) * P]
        for par in range(2):
            od = res.results[2 * j + par]["outD"].astype(np.float64)  # [8,2,128,512]
            od2 = res.results[2 * j + par]["outD2"].astype(np.float64)  # [8,128,512]
            for c in range(8):
                fo = 2 * c + par
                out_T[fo * P:(fo + 1) * P][:, tok_cols] += (
                    od[c, 0] + od[c, 1] + od2[c])
    return np.ascontiguousarray(out_T.T).reshape(1, S, H).astype(np.float32)


if __name__ == "__main__":
    nc = _build_program()
    print("build OK; instructions:",
          sum(len(b.instructions) for f in nc.m.functions for b in f.blocks))


# revision 53
# speedup vs baseline: 1.0428x; 1.0105x over previous
"""BitNet transformer block on 8 Trainium2 NeuronCores (Bass/Tile SPMD).

v3: same sharding as v2 (head-parallel attention with A2A to token-parallel,
pair TP-2 MLP), restructured for collective/compute overlap:
 - wo/xo/o_my prefetched during attention; o_proj runs as two token halves,
   half0 right after attention (A2A_lo already landed), half1 after A2A_hi.
 - ln2 + pair-AllGather chunked per token half, fired as each o_proj half
   completes; MLP gate/up weights chunk-0 prefetched during o_proj.
 - down-proj ReduceScatter in 8 chunks of 2 f-tiles; outD DMAs issued after
   the last RS trigger so the sync queue never blocks next-chunk weights.
 - engine rebalance: ln1 squares spread vector/gpsimd/scalar, MLP square on
   scalar so gpsimd stays trigger-only while collectives are in flight.
"""

import sys

import numpy as np

try:
    import concourse.bass as bass  # noqa: F401
except Exception:  # pragma: no cover
    sys.path.insert(0, "/opt/trn_rl_repo")

import ml_dtypes
import concourse.bass as bass
import concourse.mybir as mybir
import concourse.tile as tile
from concourse import bacc
from concourse.bass_utils import run_bass_kernel_spmd

FP32 = mybir.dt.float32
BF16 = mybir.dt.bfloat16
FP8 = mybir.dt.float8e4
BF = ml_dtypes.bfloat16
F8 = ml_dtypes.float8_e4m3

ALPHA = 0.7
EPS = 1e-5
NH = 16          # query heads
NKV = 4          # kv heads
D = 128          # head dim
H = 2048         # hidden
I_TOT = 8192     # mlp intermediate
S = 2048         # sequence
NC = 8           # cores
P = 128
HT = H // P      # 16 hidden tiles
HT2 = HT // 2    # 8 hidden tile-pairs (fp8 DoubleRow)
B = S // P       # 16 token blocks
I_LOC = I_TOT // 2    # 4096 intermediate per core (TP-2)
IT = I_LOC // P       # 32 inter tiles per core
TOK = 256             # tokens owned per core (2 blocks)
PTOK = 512            # tokens owned per pair
DR = mybir.MatmulPerfMode.DoubleRow

_CACHE = {}


def _build_program():
    nc = bacc.Bacc("TRN2", target_bir_lowering=False, debug=False, num_devices=NC)
    AF = mybir.ActivationFunctionType
    ALU = mybir.AluOpType
    rg_all = [list(range(NC))]
    rg_pair = [[2 * j, 2 * j + 1] for j in range(NC // 2)]

    # ---------------- inputs ----------------
    def dram_in(name, shape, dt=FP32):
        return nc.dram_tensor(name, shape, dt, kind="ExternalInput")

    xT_f = dram_in("xT_f", [P, HT, S], FP8)           # fp8 x^T ALL tokens (ln1 only)
    xT_own = dram_in("xT_own", [P, HT, TOK])          # fp32 x^T own cols (residual)
    cos_f = dram_in("cos_f", [P, S], BF16)
    sin_f = dram_in("sin_f", [P, S], BF16)
    wq_in = dram_in("wq", [P, 2, HT2, 2, P], FP8)     # my 2 heads [p,f,b,i,m]
    wk_in = dram_in("wk", [P, HT2, 2, P], FP8)        # my kv head
    wv_in = dram_in("wv", [P, HT2, 2, P], FP8)
    wo_in = dram_in("wo", [HT, P, HT, P], FP8)
    wg_in = dram_in("wg", [IT // 8, P, 8, HT2, 2, P], FP8)  # [g,p,f8,b,i,m]
    wu_in = dram_in("wu", [IT // 8, P, 8, HT2, 2, P], FP8)
    wd_in = dram_in("wd", [HT, P, IT // 2, 2, P], FP8)  # [fo,p,b,i,m]
    aq_in = dram_in("aq", [P, 2])
    ak_in = dram_in("ak", [P, 1])
    av_in = dram_in("av", [P, 1])
    ao_in = dram_in("ao", [P, HT])
    ag_in = dram_in("ag", [P, IT])
    au_in = dram_in("au", [P, IT])
    ad_in = dram_in("ad", [P, HT])
    rT_in = dram_in("rT", [P, P], BF16)               # rope rotate-half perm^T
    tril_in = dram_in("tril2", [P, TOK], BF16)        # [k, q] keep k<=q, both heads
    dmka_in = dram_in("dmka", [P, 4 * P], BF16)       # diag kb=2m: [tri|1|tri|1]
    dmkb_in = dram_in("dmkb", [P, 4 * P], BF16)       # diag kb=2m+1: [0|tri|0|tri]
    iden_in = dram_in("iden", [P, P], BF16)           # identity for PE transpose
    iden8_in = dram_in("iden8", [P, P], FP8)          # fp8 identity
    ones_f_in = dram_in("ones_f", [P, P])             # fp32 ones
    ones_b_in = dram_in("ones_b", [P, 1], BF16)       # bf16 ones column
    ones2_in = dram_in("ones2", [P, 2, 16], FP8)      # fp8 ones (DR lps lhsT)
    ones1_in = dram_in("ones1", [P, 16], FP8)         # fp8 ones (lps lhsT)
    invh_b_in = dram_in("invh_b", [P, 1], BF16)       # bf16 1/H column
    invh2_in = dram_in("invh2", [P, 2, 16], FP8)      # fp8 2^-9 (DR ssq lhsT)

    xmidT = nc.dram_tensor("xmidT", [P, HT, TOK], FP32, kind="ExternalOutput")
    outD = nc.dram_tensor("outD", [8, 2, P, PTOK], BF16, kind="ExternalOutput")
    outD2 = nc.dram_tensor("outD2", [8, P, PTOK], BF16, kind="ExternalOutput")

    a2a_lo_in = nc.dram_tensor("a2a_lo_in", [NC, P, 2, P], FP8)
    a2a_lo_out = nc.dram_tensor("a2a_lo_out", [NC, P, 2, P], FP8)
    a2a_hi_in = nc.dram_tensor("a2a_hi_in", [NC, P, 2, P], FP8)
    a2a_hi_out = nc.dram_tensor("a2a_hi_out", [NC, P, 2, P], FP8)
    # pair exchanges ride 2-rank AllGather (faster than 2-rank RS per byte).
    # down-sum, minimal wire: per-core wd/ad fo order is [mine, partner's]
    # (host permutes in_maps), slot-1 partial (partner's fo) is gathered;
    # final = (out[0]+out[1]) - sent + kept — sent cancels bit-exact, so the
    # program never needs to know its own pair parity.
    agx_in = [nc.dram_tensor(f"agx_in_{h}", [P, HT, P], FP8) for h in range(2)]
    agx_out = [nc.dram_tensor(f"agx_out_{h}", [2, P, HT, P], FP8)
               for h in range(2)]
    rs_in = [nc.dram_tensor(f"rs_in_{c}", [P, PTOK], BF16) for c in range(8)]
    rs_out = [nc.dram_tensor(f"rs_out_{c}", [2, P, PTOK], BF16)
              for c in range(8)]

    with tile.TileContext(nc) as tc:
        const = tc.alloc_tile_pool(name="const", bufs=1)
        ones_f = const.tile([P, P], FP32)
        ones_b = const.tile([P, 1], BF16)
        ones2 = const.tile([P, 2, 16], FP8)
        ones1 = const.tile([P, 16], FP8)
        invh_b = const.tile([P, 1], BF16)
        invh2 = const.tile([P, 2, 16], FP8)
        rT = const.tile([P, P], BF16)
        iden = const.tile([P, P], BF16)
        iden8 = const.tile([P, P], FP8)
        tril2 = const.tile([P, TOK], BF16)
        dmka = const.tile([P, 4 * P], BF16)
        dmkb = const.tile([P, 4 * P], BF16)
        aq = const.tile([P, 2], FP32)
        ak = const.tile([P, 1], FP32)
        av = const.tile([P, 1], FP32)
        ao = const.tile([P, HT], FP32)
        ag = const.tile([P, IT], FP32)
        au = const.tile([P, IT], FP32)
        ad = const.tile([P, HT], FP32)
        # order: tiny tiles needed by the first ssq/projection chain first;
        # late-use scale tiles (aq..ad) last so they don't delay chunk 0.
        for dst, src in [(invh2, invh2_in), (ones_f, ones_f_in), (rT, rT_in),
                         (iden, iden_in), (ones2, ones2_in), (ones1, ones1_in),
                         (ones_b, ones_b_in), (invh_b, invh_b_in),
                         (iden8, iden8_in), (tril2, tril_in),
                         (dmka, dmka_in), (dmkb, dmkb_in),
                         (aq, aq_in), (ak, ak_in), (av, av_in), (ao, ao_in),
                         (ag, ag_in), (au, au_in), (ad, ad_in)]:
            nc.sync.dma_start(dst[:], src[:])

        midpool = tc.alloc_tile_pool(name="midpool", bufs=1)
        x_mid = midpool.tile([P, HT, TOK], FP32)
        xopool = tc.alloc_tile_pool(name="xopool", bufs=1)
        xo = xopool.tile([P, HT, TOK], FP32)
        omypool = tc.alloc_tile_pool(name="omypool", bufs=1)
        o_my = omypool.tile([P, HT, TOK], FP8)      # post-A2A: 16 heads x my toks
        wores = tc.alloc_tile_pool(name="wo_res", bufs=1)
        wo_all = wores.tile([P, HT, HT, P], FP8)
        qkvpool = tc.alloc_tile_pool(name="qkvpool", bufs=1)
        q_my = qkvpool.tile([P, 2, S], BF16)         # my 2 heads, all tokens
        k_my = qkvpool.tile([P, B, P], BF16)         # my kv head [d, blk, tok]
        v_my = qkvpool.tile([P, B, P], FP8)          # my kv head [tok, blk, d]

        # ====== phase 1: ln1 (all tokens, chunked) + q/k/v TP projections ======
        CH4 = 512
        with tc.tile_pool(name="xc_pool", bufs=2) as xcp, \
             tc.tile_pool(name="p1sb", bufs=2) as p1sb, \
             tc.tile_pool(name="p1ps", bufs=1, space="PSUM") as p1ps, \
             tc.tile_pool(name="p2ps", bufs=2, space="PSUM") as p2ps, \
             tc.tile_pool(name="rot_ps", bufs=2, space="PSUM") as rot_ps, \
             tc.tile_pool(name="vt_ps", bufs=2, space="PSUM") as vt_ps, \
             tc.tile_pool(name="p2sb", bufs=2) as p2sb, \
             tc.tile_pool(name="cs_pool", bufs=2) as csp, \
             tc.tile_pool(name="wres", bufs=1) as wres:
            # chunk-0 x lands before the projection weights: the ssq square
            # chain (vector/gpsimd/scalar) only needs x, weights are needed
            # a few microseconds later at the first q matmul.
            wq_sb = wres.tile([P, 2, HT2, 2, P], FP8)
            wk_sb = wres.tile([P, HT2, 2, P], FP8)
            wv_sb = wres.tile([P, HT2, 2, P], FP8)
            for c4 in range(4):
                tsl = slice(c4 * CH4, (c4 + 1) * CH4)
                xc = xcp.tile([P, HT, CH4], FP8, name="xc")
                # split the 1MB chunk across both DMA queues so the first
                # hidden tiles land sooner (the ssq chain consumes in order)
                nc.scalar.dma_start(xc[:, 0:HT // 2, :], xT_f[:, 0:HT // 2, tsl])
                nc.sync.dma_start(xc[:, HT // 2:, :], xT_f[:, HT // 2:, tsl])
                if c4 == 0:
                    nc.scalar.dma_start(wq_sb[:], wq_in[:])
                    nc.scalar.dma_start(wk_sb[:], wk_in[:])
                    nc.scalar.dma_start(wv_sb[:], wv_in[:])
                cfc = csp.tile([P, CH4], BF16, name="cfc")
                nc.sync.dma_start(cfc[:], cos_f[:, tsl])
                sfc = csp.tile([P, CH4], BF16, name="sfc")
                nc.sync.dma_start(sfc[:], sin_f[:, tsl])
                # rmsnorm scale from fp8 x; applied per-token at PSUM evict.
                # squares spread across vector/gpsimd/scalar (DVE was the
                # phase-1 co-bottleneck).
                ssq = p1ps.tile([16, CH4], FP32, name="ssq")
                for b in range(HT2):
                    sq2 = p1sb.tile([P, 2, CH4], FP8, name="sq2")
                    for j in range(2):
                        k = (2 * b + j) % 3
                        if k == 0:
                            nc.vector.tensor_mul(sq2[:, j, :], xc[:, 2 * b + j, :],
                                                 xc[:, 2 * b + j, :])
                        elif k == 1:
                            nc.gpsimd.tensor_mul(sq2[:, j, :], xc[:, 2 * b + j, :],
                                                 xc[:, 2 * b + j, :])
                        else:
                            nc.scalar.activation(sq2[:, j, :], xc[:, 2 * b + j, :],
                                                 AF.Square)
                    nc.tensor.matmul(ssq[:], invh2[:], sq2[:],
                                     start=(b == 0), stop=(b == HT2 - 1),
                                     perf_mode=DR)
                ssb = p1sb.tile([1, CH4], FP32, name="ssb")
                nc.scalar.activation(ssb[:], ssq[0:1, :], AF.Copy)
                msb = p1ps.tile([P, CH4], FP32, name="msb")
                nc.tensor.matmul(msb[:], ones_f[0:1, :], ssb[:],
                                 start=True, stop=True)
                rec = p1sb.tile([P, CH4], FP32, name="rec")
                nc.vector.reciprocal_approx_fast(rec[:], msb[:])
                rsq_bc = p1sb.tile([P, CH4], FP32, name="rsq_bc", tag="rsq_bc")
                nc.scalar.activation(rsq_bc[:], rec[:], AF.Sqrt, scale=4.0)
                # q (2 heads): both PSUM chains first, then both rotations,
                # then the elementwise rope stage — avoids the per-head
                # PE->vector->PE ping-pong (head-of-line stalls on both).
                qs2 = [None, None]
                for f in range(2):
                    ps = p2ps.tile([P, CH4], FP32, name="pps")
                    for b in range(HT2):
                        nc.tensor.matmul(ps[:], wq_sb[:, f, b, :, :],
                                         xc[:, 2 * b:2 * b + 2, :],
                                         start=(b == 0), stop=(b == HT2 - 1),
                                         perf_mode=DR)
                    qs = p2sb.tile([P, CH4], BF16, name=f"qs{f}")
                    nc.vector.scalar_tensor_tensor(qs[:], ps[:], aq[:, f:f + 1],
                                                   rsq_bc[:], ALU.mult, ALU.mult)
                    qs2[f] = qs
                rot2 = [None, None]
                for f in range(2):
                    rot = rot_ps.tile([P, CH4], FP32, name="rot")
                    nc.tensor.matmul(rot[:], rT[:], qs2[f][:],
                                     start=True, stop=True)
                    rot2[f] = rot
                for f in range(2):
                    t2 = p2sb.tile([P, CH4], BF16, name="t2")
                    nc.vector.tensor_mul(t2[:], qs2[f][:], cfc[:])
                    t1 = p2sb.tile([P, CH4], BF16, name="t1")
                    nc.vector.tensor_mul(t1[:], rot2[f][:], sfc[:])
                    nc.gpsimd.tensor_add(q_my[:, f, tsl], t1[:], t2[:])
                # k: my kv head
                ps = p2ps.tile([P, CH4], FP32, name="pps")
                for b in range(HT2):
                    nc.tensor.matmul(ps[:], wk_sb[:, b, :, :],
                                     xc[:, 2 * b:2 * b + 2, :],
                                     start=(b == 0), stop=(b == HT2 - 1),
                                     perf_mode=DR)
                ks = p2sb.tile([P, CH4], BF16, name="qs0")
                nc.vector.scalar_tensor_tensor(ks[:], ps[:], ak[:, 0:1],
                                               rsq_bc[:], ALU.mult, ALU.mult)
                rot = rot_ps.tile([P, CH4], FP32, name="rot")
                nc.tensor.matmul(rot[:], rT[:], ks[:], start=True, stop=True)
                t2 = p2sb.tile([P, CH4], BF16, name="t2")
                nc.vector.tensor_mul(t2[:], ks[:], cfc[:])
                t1 = p2sb.tile([P, CH4], BF16, name="t1")
                nc.vector.tensor_mul(t1[:], rot[:], sfc[:])
                nc.gpsimd.tensor_add(
                    k_my[:, 4 * c4:4 * c4 + 4, :].rearrange("p b t -> p (b t)"),
                    t1[:], t2[:])
                # v: my kv head, then PE-transpose to [tok, d] (fp8)
                ps = p2ps.tile([P, CH4], FP32, name="pps")
                for b in range(HT2):
                    nc.tensor.matmul(ps[:], wv_sb[:, b, :, :],
                                     xc[:, 2 * b:2 * b + 2, :],
                                     start=(b == 0), stop=(b == HT2 - 1),
                                     perf_mode=DR)
                vtv = p2sb.tile([P, CH4], BF16, name="vtv")
                nc.vector.scalar_tensor_tensor(vtv[:], ps[:], av[:, 0:1],
                                               rsq_bc[:], ALU.mult, ALU.mult)
                for j in range(4):
                    vtp = vt_ps.tile([P, P], BF16, name="vtp")
                    nc.tensor.transpose(vtp[:], vtv[:, j * P:(j + 1) * P], iden[:])
                    nc.vector.tensor_copy(v_my[:, 4 * c4 + j, :], vtp[:])

        # ========= prefetch for phase 3 (overlaps attention on DMA) =========
        for g in range(4):
            eng = nc.sync if g % 2 == 0 else nc.scalar
            eng.dma_start(wo_all[:, 4 * g:4 * (g + 1), :, :],
                          wo_in[4 * g:4 * (g + 1)].rearrange("f p k m -> p f k m"))
        nc.sync.dma_start(xo[:], xT_own[:])

        # ====== phase 2: attention (two consecutive query blocks fused) ======
        # q blocks {2m, 2m+1} processed together: FD=512 matmuls (columns
        # [h, qb, t]); the diagonal kb pair uses mask constants
        # dmka = [tri|1|tri|1] (kb=2m), dmkb = [0|tri|0|tri] (kb=2m+1).
        with tc.tile_pool(name="a_ps", bufs=2, space="PSUM") as a_ps, \
             tc.tile_pool(name="o_ps", bufs=2, space="PSUM") as o_ps, \
             tc.tile_pool(name="lbc_ps", bufs=1, space="PSUM") as lbc_ps, \
             tc.tile_pool(name="a_sb", bufs=3) as a_sb:
            for m in range(8):
                qb0 = 2 * m
                q_pair = q_my[:, :, qb0 * P:(qb0 + 2) * P]   # [P, 2, 256]
                ops = o_ps.tile([P, 2, 2, P], FP32, name="ops")   # [d][h][q][t]
                lps = lbc_ps.tile([16, 4 * P], FP32, name="lps")
                for g in range(m + 1):
                    kb0 = 2 * g
                    first, diag = g == 0, g == m
                    sps = a_ps.tile([P, 2, 4 * P], FP32, name="sps")
                    for j in range(2):
                        nc.tensor.matmul(sps[:, j, :], k_my[:, kb0 + j, :],
                                         q_pair, start=True, stop=True)
                    pm2 = a_sb.tile([P, 2, 4 * P], FP8, name="pm2")
                    if diag:
                        pmd = a_sb.tile([P, 2, 4 * P], BF16, name="pmd")
                        nc.scalar.activation(
                            pmd[:].rearrange("p a t -> p (a t)"),
                            sps[:].rearrange("p a t -> p (a t)"), AF.Exp)
                        nc.vector.tensor_mul(pm2[:, 0, :], pmd[:, 0, :], dmka[:])
                        nc.vector.tensor_mul(pm2[:, 1, :], pmd[:, 1, :], dmkb[:])
                    else:
                        nc.scalar.activation(
                            pm2[:].rearrange("p a t -> p (a t)"),
                            sps[:].rearrange("p a t -> p (a t)"), AF.Exp)
                    nc.tensor.matmul(lps[:], ones2[:], pm2[:],
                                     start=first, stop=diag, perf_mode=DR)
                    nc.tensor.matmul(ops[:].rearrange("p h q t -> p (h q t)"),
                                     v_my[:, kb0:kb0 + 2, :], pm2[:],
                                     start=first, stop=diag, perf_mode=DR)
                lsb = a_sb.tile([1, 4 * P], FP32, name="lsb")
                nc.scalar.activation(lsb[:], lps[0:1, :], AF.Copy)
                bca = lbc_ps.tile([P, 4 * P], FP32, name="bca")
                nc.tensor.matmul(bca[:], ones_f[0:1, :], lsb[:],
                                 start=True, stop=True)
                linv = a_sb.tile([P, 4 * P], FP32, name="linv")
                nc.vector.reciprocal_approx_fast(linv[:], bca[:])
                osb = a_sb.tile([P, 2, 2, P], FP8, name="osb")
                nc.vector.tensor_mul(
                    osb[:].rearrange("p h q t -> p (h q t)"),
                    ops[:].rearrange("p h q t -> p (h q t)"), linv[:])
                for j in range(2):
                    qb = qb0 + j
                    r_dst = min(qb, 15 - qb)
                    dst = a2a_lo_in if qb < 8 else a2a_hi_in
                    nc.sync.dma_start(dst[r_dst][:], osb[:, :, j, :])
                if m == 3:
                    nc.gpsimd.collective_compute(
                        "AllToAll", ALU.bypass, ins=[a2a_lo_in[:]],
                        outs=[a2a_lo_out[:]], replica_groups=rg_all)
                if m == 5:
                    # A2A_lo has landed by now; pull the low-half heads in
                    # while the tail query blocks compute.
                    for j in range(NC):
                        nc.sync.dma_start(o_my[:, 2 * j:2 * j + 2, 0:P],
                                          a2a_lo_out[j])
            nc.gpsimd.collective_compute(
                "AllToAll", ALU.bypass, ins=[a2a_hi_in[:]],
                outs=[a2a_hi_out[:]], replica_groups=rg_all)
        qkvpool.release()

        # ===== phase 3: o_proj + residual + ln2 (token halves) + pair-AG =====
        # MLP pools allocated early so gate/up chunk 0 prefetches during o_proj
        h2cp = tc.alloc_tile_pool(name="h2c_pool", bufs=1)
        h2c = h2cp.tile([P, HT, PTOK], FP8)
        mp = tc.alloc_tile_pool(name="m_pool", bufs=1)
        m_all = mp.tile([P, IT, PTOK], FP8)
        wgup = tc.alloc_tile_pool(name="wgu_pool", bufs=2)
        wtg = [None] * 4
        wtu = [None] * 4
        wtg[0] = wgup.tile([P, 8, HT2, 2, P], FP8, name="wtg8")
        nc.sync.dma_start(wtg[0][:], wg_in[0])
        wtu[0] = wgup.tile([P, 8, HT2, 2, P], FP8, name="wtu8")
        nc.scalar.dma_start(wtu[0][:], wu_in[0])

        with tc.tile_pool(name="p5ps", bufs=2, space="PSUM") as p5ps, \
             tc.tile_pool(name="p5sb", bufs=3) as p5sb, \
             tc.tile_pool(name="h2h_pool", bufs=1) as h2hp:
            h2h = h2hp.tile([P, HT, TOK], FP8)
            # high-half heads: sync queue waits on A2A_hi while PE runs half 0
            for j in range(NC):
                nc.sync.dma_start(o_my[:, 2 * j:2 * j + 2, P:TOK],
                                  a2a_hi_out[j])
            ssq2 = p5ps.tile([16, TOK], FP32, name="ssq2")
            for half in range(2):
                csl = slice(half * P, (half + 1) * P)
                # ln2 sum-of-squares interleaved into the o_proj f-loop at
                # pair granularity so the PE never drains waiting on the
                # DVE/ACT square chain after the last f-tile.
                for f in range(HT):
                    ps = p5ps.tile([P, P], FP32, name="ops5")
                    for kt in range(HT):
                        nc.tensor.matmul(ps[:], wo_all[:, f, kt, :], o_my[:, kt, csl],
                                         start=(kt == 0), stop=(kt == HT - 1))
                    nc.vector.scalar_tensor_tensor(
                        x_mid[:, f, csl], ps[:], ao[:, f:f + 1],
                        xo[:, f, csl], ALU.mult, ALU.add)
                    if f % 2 == 1:
                        b = f // 2
                        sq2 = p5sb.tile([P, 2, P], FP8, name="sq2h")
                        nc.vector.tensor_mul(sq2[:, 0, :],
                                             x_mid[:, 2 * b, csl],
                                             x_mid[:, 2 * b, csl])
                        nc.scalar.activation(sq2[:, 1, :],
                                             x_mid[:, 2 * b + 1, csl],
                                             AF.Square)
                        nc.tensor.matmul(ssq2[:, csl], invh2[:], sq2[:],
                                         start=(b == 0), stop=(b == HT2 - 1),
                                         perf_mode=DR)
                ssb2 = p5sb.tile([1, P], FP32, name="ssb2")
                nc.scalar.activation(ssb2[:], ssq2[0:1, csl], AF.Copy)
                msb2 = p5ps.tile([P, P], FP32, name="msb2")
                nc.tensor.matmul(msb2[:], ones_f[0:1, :], ssb2[:],
                                 start=True, stop=True)
                rec2 = p5sb.tile([P, P], FP32, name="rec2")
                nc.vector.reciprocal_approx_fast(rec2[:], msb2[:])
                rsq2 = p5sb.tile([P, P], FP32, name="rsq2")
                nc.scalar.activation(rsq2[:], rec2[:], AF.Sqrt, scale=4.0)
                for kt in range(HT):
                    # gpsimd only before its first trigger (AG0) is enqueued:
                    # a trigger may block the queue until the CC completes.
                    eng = (nc.gpsimd if (half == 0 and kt % 2 == 1)
                           else nc.vector)
                    eng.tensor_mul(h2h[:, kt, csl], x_mid[:, kt, csl],
                                   rsq2[:])
                nc.sync.dma_start(agx_in[half][:], h2h[:, :, csl])
                nc.gpsimd.collective_compute(
                    "AllGather", ALU.bypass, ins=[agx_in[half][:]],
                    outs=[agx_out[half][:]], replica_groups=rg_pair)
                nc.scalar.dma_start(xmidT[:, :, csl], x_mid[:, :, csl])

        # ========== phase 4: MLP (pair TP-2 over inter) + chunked RS ==========
        with tc.tile_pool(name="wd_pool", bufs=4) as wdp, \
             tc.tile_pool(name="p7ps", bufs=2, space="PSUM") as p7ps, \
             tc.tile_pool(name="p7dps", bufs=2, space="PSUM") as p7dps, \
             tc.tile_pool(name="p7sb", bufs=4) as p7sb, \
             tc.tile_pool(name="rsum", bufs=2) as rsum:
            # pair token order: [2j's 256 | (2j+1)'s 256], each = [lo128|hi128]
            for half in range(2):
                for r in range(2):
                    eng = nc.sync if r == 0 else nc.scalar
                    eng.dma_start(
                        h2c[:, :, r * TOK + half * P:r * TOK + (half + 1) * P],
                        agx_out[half][r])
            for f in range(IT):
                if f % 8 == 0 and f // 8 + 1 < 4:
                    k = f // 8 + 1
                    wtg[k] = wgup.tile([P, 8, HT2, 2, P], FP8, name="wtg8")
                    nc.sync.dma_start(wtg[k][:], wg_in[k])
                    wtu[k] = wgup.tile([P, 8, HT2, 2, P], FP8, name="wtu8")
                    nc.scalar.dma_start(wtu[k][:], wu_in[k])
                wtg8, wtu8 = wtg[f // 8], wtu[f // 8]
                gps = p7ps.tile([P, PTOK], FP32, name="gps")
                for b in range(HT2):
                    nc.tensor.matmul(gps[:], wtg8[:, f % 8, b, :, :],
                                     h2c[:, 2 * b:2 * b + 2, :],
                                     start=(b == 0), stop=(b == HT2 - 1),
                                     perf_mode=DR)
                ups = p7ps.tile([P, PTOK], FP32, name="ups")
                for b in range(HT2):
                    nc.tensor.matmul(ups[:], wtu8[:, f % 8, b, :, :],
                                     h2c[:, 2 * b:2 * b + 2, :],
                                     start=(b == 0), stop=(b == HT2 - 1),
                                     perf_mode=DR)
                gr = p7sb.tile([P, PTOK], BF16, name="gr")
                nc.vector.tensor_scalar(gr[:], gps[:], ag[:, f:f + 1], 0.0,
                                        ALU.mult, ALU.max)
                g2 = p7sb.tile([P, PTOK], BF16, name="g2")
                nc.scalar.activation(g2[:], gr[:], AF.Square)
                nc.vector.scalar_tensor_tensor(m_all[:, f, :], ups[:],
                                               au[:, f:f + 1], g2[:],
                                               ALU.mult, ALU.mult)
            # down proj in 8 chunks of 2 fo; slot 0 = MY fo (kept local),
            # slot 1 = partner's fo (gathered). dd = kept - sent goes out via
            # outD2 off the critical path; the gathered slots go out raw via
            # a single dram->dram DMA per chunk (host does the 3-way add in
            # f64: fin = out[0] + out[1] + dd — the sent tile cancels).
            def consume_rs(c):
                eng = nc.sync if c % 2 == 0 else nc.scalar
                eng.dma_start(outD[c], rs_out[c][:])

            for c in range(8):
                dn2 = [None, None]
                for j in range(2):
                    fo = 2 * c + j
                    wtd = wdp.tile([P, IT // 2, 2, P], FP8, name="wtd")
                    nc.sync.dma_start(wtd[:], wd_in[fo])
                    dps = p7dps.tile([P, PTOK], FP32, name="dps")
                    for b in range(IT // 2):
                        nc.tensor.matmul(dps[:], wtd[:, b, :, :],
                                         m_all[:, 2 * b:2 * b + 2, :],
                                         start=(b == 0), stop=(b == IT // 2 - 1),
                                         perf_mode=DR)
                    dn = p7sb.tile([P, PTOK], BF16, name=f"dn{j}")
                    nc.scalar.activation(dn[:], dps[:], AF.Copy,
                                         scale=ad[:, fo:fo + 1])
                    dn2[j] = dn
                nc.sync.dma_start(rs_in[c][:], dn2[1][:])
                dd = p7sb.tile([P, PTOK], BF16, name="dd")
                nc.vector.tensor_sub(dd[:], dn2[0][:], dn2[1][:])
                nc.scalar.dma_start(outD2[c], dd[:])
                nc.gpsimd.collective_compute(
                    "AllGather", ALU.bypass, ins=[rs_in[c][:]],
                    outs=[rs_out[c][:]], replica_groups=rg_pair)
                if c >= 2:
                    consume_rs(c - 2)
            consume_rs(6)
            consume_rs(7)
        wgup.release()
        mp.release()
        h2cp.release()
        wores.release()
        omypool.release()
        xopool.release()
        midpool.release()
        const.release()

    nc.finalize()
    return nc


def _ternary(w, fold_row=None):
    """Quantize [O, Hin] fp32 -> (ternary fp32 {-1,0,1}, absmean [O])."""
    w = np.asarray(w, dtype=np.float32)
    am = np.mean(np.abs(w), axis=1)
    t = np.sign(w) * (np.abs(w) > ALPHA * am[:, None]).astype(np.float32)
    if fold_row is not None:
        t = t * fold_row[None, :]
    return t, am


def _wlhsT(tern, n_f):
    """ternary [O, Hin] -> bf16 lhsT layout [f, p, kt, c]."""
    o, hin = tern.shape
    kt = hin // P
    assert n_f * P == o
    wT = np.ascontiguousarray(tern.T)  # [Hin, O]
    return np.ascontiguousarray(
        wT.reshape(kt, P, n_f, P).transpose(2, 1, 0, 3)).astype(BF)


def _wlhsT_dr(tern, n_f):
    """ternary [O, Hin] -> fp8 DoubleRow lhsT layout [p, f, b, i, m]:
    w[p, f, b, i, m] = ternT[128*(2b+i)+p, 128*f+m]."""
    o, hin = tern.shape
    b2 = hin // (2 * P)
    assert n_f * P == o
    wT = np.ascontiguousarray(tern.T)  # [Hin, O]
    return np.ascontiguousarray(
        wT.reshape(b2, 2, P, n_f, P).transpose(2, 3, 0, 1, 4)).astype(F8)


def _wd_layout(td_slice):
    """[H, I_loc] -> fp8 DoubleRow [fo, p, b, i, m]: wd[fo, p, b, i, m] =
    td_slice[128*fo+m, 128*(2b+i)+p]."""
    hin, iloc = td_slice.shape
    assert hin == H and iloc == I_LOC
    wT = np.ascontiguousarray(td_slice.T)  # [I_loc, H]
    return np.ascontiguousarray(
        wT.reshape(IT // 2, 2, P, HT, P).transpose(3, 2, 0, 1, 4)).astype(F8)


def _scale_tiles(a):
    """[O] -> [P, O//P] with column f = features f*128..f*128+127."""
    return np.ascontiguousarray(a.reshape(-1, P).T).astype(np.float32)


def _pcol(x2d):
    """[K, T] -> [P, K//P, T] (partition-major for direct DMA)."""
    k, t = x2d.shape
    return np.ascontiguousarray(
        x2d.reshape(k // P, P, t).transpose(1, 0, 2)).astype(np.float32)


def kernel(x, cos, sin, wq, wk, wv, wo, wg, wu, wd, ln1_w, ln2_w):
    x = np.asarray(x, dtype=np.float32)
    b, s, hdim = x.shape
    assert (b, s, hdim) == (1, S, H)

    if "nc" not in _CACHE:
        _CACHE["nc"] = _build_program()
    nc = _CACHE["nc"]

    ln1 = np.asarray(ln1_w, dtype=np.float32)
    ln2 = np.asarray(ln2_w, dtype=np.float32)

    tq, amq = _ternary(wq, fold_row=ln1)
    tk, amk = _ternary(wk, fold_row=ln1)
    tv, amv = _ternary(wv, fold_row=ln1)
    to, amo = _ternary(wo)
    tg, amg = _ternary(wg, fold_row=ln2)
    tu, amu = _ternary(wu, fold_row=ln2)
    td, amd = _ternary(wd)

    wq_h = _wlhsT_dr(tq, NH)          # [P, 16, 8, 2, P]
    wk_h = _wlhsT_dr(tk, NKV)         # [P, 4, 8, 2, P]
    wv_h = _wlhsT_dr(tv, NKV)
    wo_h = _wlhsT(to, HT).astype(F8)  # [16, P, 16, P] fp8

    aq_h = _scale_tiles(amq / np.sqrt(np.float32(D)))
    ak_h = _scale_tiles(amk)
    av_h = _scale_tiles(amv)
    ao_h = _scale_tiles(amo)
    ag_h = _scale_tiles(amg)          # [P, 64]
    au_h = _scale_tiles(amu)
    ad_h = _scale_tiles(amd)          # [P, 16]

    x2 = x[0]
    xT = np.ascontiguousarray(x2.T)
    xT_f = _pcol(xT)
    cosT = np.ascontiguousarray(np.asarray(cos, np.float32)[0, 0].T).astype(BF)
    sinT = np.ascontiguousarray(np.asarray(sin, np.float32)[0, 0].T).astype(BF)

    R = np.zeros((P, P), np.float32)
    for m in range(64):
        R[m, m + 64] = -1.0
        R[m + 64, m] = 1.0
    rT_h = np.ascontiguousarray(R.T).astype(BF)
    ones_f = np.ones((P, P), np.float32)
    ones_b = np.ones((P, 1), np.float32).astype(BF)
    invh_b = np.full((P, 1), 1.0 / H, np.float32).astype(BF)
    ones2_h = np.ones((P, 2, 16), np.float32).astype(F8)
    invh2_h = np.full((P, 2, 16), 2.0 ** -9, np.float32).astype(F8)
    ones1_h = np.ones((P, 16), np.float32).astype(F8)
    triu = np.triu(np.ones((P, P), np.float32))
    tril2_h = np.ascontiguousarray(np.concatenate([triu, triu], axis=1)).astype(BF)
    onep = np.ones((P, P), np.float32)
    zep = np.zeros((P, P), np.float32)
    dmka_h = np.ascontiguousarray(
        np.concatenate([triu, onep, triu, onep], axis=1)).astype(BF)
    dmkb_h = np.ascontiguousarray(
        np.concatenate([zep, triu, zep, triu], axis=1)).astype(BF)
    iden_h = np.eye(P, dtype=np.float32).astype(BF)

    in_maps = []
    for i in range(NC):
        blo, bhi = i, 15 - i
        own_cols = np.r_[blo * P:(blo + 1) * P, bhi * P:(bhi + 1) * P]
        kvh = i // 2
        par = i % 2
        isl = slice(par * IT, (par + 1) * IT)       # inter tile slice (TP-2)
        irow = slice(par * I_LOC, (par + 1) * I_LOC)
        # down-proj fo slot order per core: [mine (2c+par), partner's]
        fo_perm = [2 * c + (j ^ par) for c in range(8) for j in range(2)]
        in_maps.append({
            "xT_f": xT_f.astype(F8),
            "xT_own": _pcol(xT[:, own_cols]),
            "cos_f": cosT, "sin_f": sinT,
            "wq": np.ascontiguousarray(wq_h[:, 2 * i:2 * i + 2]),
            "wk": np.ascontiguousarray(wk_h[:, kvh]),
            "wv": np.ascontiguousarray(wv_h[:, kvh]),
            "wo": wo_h,
            "wg": np.ascontiguousarray(
                _wlhsT_dr(tg[irow], IT).reshape(P, 4, 8, HT2, 2, P)
                .transpose(1, 0, 2, 3, 4, 5)),
            "wu": np.ascontiguousarray(
                _wlhsT_dr(tu[irow], IT).reshape(P, 4, 8, HT2, 2, P)
                .transpose(1, 0, 2, 3, 4, 5)),
            "wd": np.ascontiguousarray(_wd_layout(td[:, irow])[fo_perm]),
            "aq": np.ascontiguousarray(aq_h[:, 2 * i:2 * i + 2]),
            "ak": np.ascontiguousarray(ak_h[:, kvh:kvh + 1]),
            "av": np.ascontiguousarray(av_h[:, kvh:kvh + 1]),
            "ao": ao_h,
            "ag": np.ascontiguousarray(ag_h[:, isl]),
            "au": np.ascontiguousarray(au_h[:, isl]),
            "ad": np.ascontiguousarray(ad_h[:, fo_perm]),
            "rT": rT_h, "tril2": tril2_h, "dmka": dmka_h, "dmkb": dmkb_h,
            "iden": iden_h,
            "iden8": iden_h.astype(F8),
            "ones_f": ones_f, "ones_b": ones_b, "invh_b": invh_b,
            "ones2": ones2_h, "ones1": ones1_h, "invh2": invh2_h,
        })

    res = run_bass_kernel_spmd(nc, in_maps, list(range(NC)))
    _CACHE["last_result"] = res

    # ---- host-side unshard: xmid residual + pair-RS output assembly ----
    out_T = np.zeros((H, S), np.float64)
    for i in range(NC):
        blo, bhi = i, 15 - i
        xm = res.results[i]["xmidT"].astype(np.float64)      # [P, HT, 256]
        xm = xm.transpose(1, 0, 2).reshape(H, TOK)
        out_T[:, blo * P:(blo + 1) * P] += xm[:, 0:P]
        out_T[:, bhi * P:(bhi + 1) * P] += xm[:, P:TOK]
    for j in range(NC // 2):
        # pair token order: [core 2j's 256 | core 2j+1's 256]
        tok_cols = np.r_[(2 * j) * P:(2 * j + 1) * P,
                         (15 - 2 * j) * P:(16 - 2 * j) * P,
                         (2 * j + 1) * P:(2 * j + 2) * P,
                         (14 - 2 * j) * P:(15 - 2 * j) * P]
        for par in range(2):
            od = res.results[2 * j + par]["outD"].astype(np.float64)  # [8,2,128,512]
            od2 = res.results[2 * j + par]["outD2"].astype(np.float64)  # [8,128,512]
            for c in range(8):
                fo = 2 * c + par
                out_T[fo * P:(fo + 1) * P][:, tok_cols] += (
                    od[c, 0] + od[c, 1] + od2[c])
    return np.ascontiguousarray(out_T.T).reshape(1, S, H).astype(np.float32)


if __name__ == "__main__":
    nc = _build_program()
    print("build OK; instructions:",
          sum(len(b.instructions) for f in nc.m.functions for b in f.blocks))
